# revision 1
# baseline (speedup 1.0000x reference)
"""Trainium2 Bass kernel for nn_Decoder (2-layer bidirectional LSTM decoder,
autoregressive argmax feedback, T=512 steps, B=128, H=1024, V=64).

Strategy: 8-way tensor parallel over the 4H gate dimension. Each core holds a
512-wide slice of every gate projection (re-ordered [i,f,o,g] so activations
fuse), keeps the LSTM recurrence state resident, and exchanges the 128-row
h-slices it owns via two AllGathers per step. Matmuls run as fp32r (TF32) with
the h-state (transposed) as the stationary operand and the weight slice as the
512-wide moving operand. Logits are computed as per-core partials, summed
after the second AllGather, and the argmax feeds the next step on-device.
"""

import os
import sys

import numpy as np

sys.path.insert(0, "/opt/trn_rl_repo")

import concourse.bass as bass  # noqa: E402
import concourse.mybir as mybir  # noqa: E402
import concourse.tile as tile  # noqa: E402
from concourse import bacc  # noqa: E402
from concourse import bass_utils  # noqa: E402
from concourse.masks import make_identity  # noqa: E402

H = 1024
V = 64
B = 128
NCORES = 8
MASK_IDX = 4.0
KEEP_IDX = 3
T_STEPS = int(os.environ.get("DEC_T", "512"))
CHUNK = int(os.environ.get("DEC_CHUNK", "16"))
MM_DT = mybir.dt.float32r if os.environ.get("DEC_MMDT", "fp32r") == "fp32r" else mybir.dt.float32
F32 = mybir.dt.float32
MMD = MM_DT
AF = mybir.ActivationFunctionType
ALU = mybir.AluOpType

# gate blocks packed per-core as [i, f, o, g] (torch order in rows is i,f,g,o)
GBASE = [0, H, 3 * H, 2 * H]


def tf32_round(x):
    if MM_DT == F32:
        return np.asarray(x, np.float32)
    xi = np.asarray(x, np.float32).view(np.uint32)
    xi = (xi + np.uint32(1 << 12)) & np.uint32(0xFFFFE000)
    return xi.view(np.float32)


def build(T=T_STEPS):
    nc = bacc.Bacc("TRN2", num_devices=NCORES)
    RG = [list(range(NCORES))]

    din = dict(kind="ExternalInput")
    w0T = nc.dram_tensor("w0T", [2, 128, 8, 512], MMD, **din)
    w0aug = nc.dram_tensor("w0aug", [2, 2, 512], MMD, **din)
    w1iT = nc.dram_tensor("w1iT", [2, 128, 16, 512], MMD, **din)
    w1hT = nc.dram_tensor("w1hT", [2, 128, 8, 512], MMD, **din)
    b1row = nc.dram_tensor("b1row", [2, 1, 512], MMD, **din)
    linTc = nc.dram_tensor("linTc", [128, 2, 64], MMD, **din)
    linb = nc.dram_tensor("linb", [128, 64], F32, **din)
    iotam = nc.dram_tensor("iotam", [128, 64], F32, **din)
    notkeep = nc.dram_tensor("notkeep", [128, 64], F32, **din)
    hT0 = nc.dram_tensor("hT0", [4, 128, 8, 128], MMD, **din)
    c0s = nc.dram_tensor("c0s", [4, 128, 128], F32, **din)
    onesrow = nc.dram_tensor("onesrow", [1, 128], MMD, **din)
    x0row = nc.dram_tensor("x0row", [1, 128], MMD, **din)
    flag0 = nc.dram_tensor("flag0", [128, 1], F32, **din)
    hT_f = nc.dram_tensor("hT_f", [4, 128, 8, 128], MMD, kind="ExternalOutput")
    c_f = nc.dram_tensor("c_f", [4, 128, 128], F32, kind="ExternalOutput")
    flag_f = nc.dram_tensor("flag_f", [128, 1], F32, kind="ExternalOutput")
    idx_f = nc.dram_tensor("idx_f", [128, 1], F32, kind="ExternalOutput")
    y = nc.dram_tensor("y", [B, T, V], F32, kind="ExternalOutput")

    with tile.TileContext(nc) as tc:
        import contextlib

        ctx = contextlib.ExitStack()
        with ctx:
            wp = ctx.enter_context(tc.tile_pool(name="weights", bufs=1))
            hp = ctx.enter_context(tc.tile_pool(name="hstate", bufs=2))
            cp = ctx.enter_context(tc.tile_pool(name="cstate", bufs=2))
            gp = ctx.enter_context(tc.tile_pool(name="gact", bufs=1))
            ewp = ctx.enter_context(tc.tile_pool(name="ew", bufs=2))
            sp = ctx.enter_context(tc.tile_pool(name="send", bufs=2))
            ap_ = ctx.enter_context(tc.tile_pool(name="amax", bufs=2))
            yp = ctx.enter_context(tc.tile_pool(name="ybuf", bufs=2))
            pg = ctx.enter_context(tc.tile_pool(name="pgates", bufs=1, space="PSUM"))
            pt = ctx.enter_context(tc.tile_pool(name="ptrans", bufs=2, space="PSUM"))
            px = ctx.enter_context(tc.tile_pool(name="pmisc", bufs=1, space="PSUM"))
            dp = ctx.enter_context(tc.tile_pool(name="dram", bufs=2, space="DRAM"))

            # ---- load weights + constants (once) ----
            w0_sb, w0a_sb, w1i_sb, w1h_sb, b1_sb = [], [], [], [], []
            for d in range(2):
                t_ = wp.tile([128, 8, 512], MMD, tag=f"w0_{d}")
                nc.sync.dma_start(out=t_[:], in_=w0T[d])
                w0_sb.append(t_)
                tb = wp.tile([1, 512], MMD, tag=f"w0b_{d}")
                nc.sync.dma_start(out=tb[:], in_=w0aug[d, 1:2])
                tx = wp.tile([1, 512], MMD, tag=f"w0x_{d}")
                nc.sync.dma_start(out=tx[:], in_=w0aug[d, 0:1])
                w0a_sb.append((tx, tb))
                t_ = wp.tile([128, 16, 512], MMD, tag=f"w1i_{d}")
                nc.sync.dma_start(out=t_[:], in_=w1iT[d])
                w1i_sb.append(t_)
                t_ = wp.tile([128, 8, 512], MMD, tag=f"w1h_{d}")
                nc.sync.dma_start(out=t_[:], in_=w1hT[d])
                w1h_sb.append(t_)
                t_ = wp.tile([1, 512], MMD, tag=f"b1_{d}")
                nc.sync.dma_start(out=t_[:], in_=b1row[d])
                b1_sb.append(t_)
            lin_sb = wp.tile([128, 2, 64], MMD, tag="lin")
            nc.sync.dma_start(out=lin_sb[:], in_=linTc[:])
            linb_sb = wp.tile([128, 64], F32, tag="linb")
            nc.sync.dma_start(out=linb_sb[:], in_=linb[:])
            iot_sb = wp.tile([128, 64], F32, tag="iot")
            nc.sync.dma_start(out=iot_sb[:], in_=iotam[:])
            nk_sb = wp.tile([128, 64], F32, tag="nk")
            nc.sync.dma_start(out=nk_sb[:], in_=notkeep[:])
            ident = wp.tile([128, 128], F32, tag="ident")
            make_identity(nc, ident[:])
            ones = wp.tile([1, 128], MMD, tag="ones")
            nc.sync.dma_start(out=ones[:], in_=onesrow[:])

            # ---- initial state ----
            h_prev = []
            for cell in range(4):
                t_ = hp.tile([128, 8, 128], MMD, tag=f"h{cell}")
                nc.sync.dma_start(out=t_[:], in_=hT0[cell])
                h_prev.append(t_)
            c_prev = []
            for cell in range(4):
                t_ = cp.tile([128, 128], F32, tag=f"c{cell}")
                nc.sync.dma_start(out=t_[:], in_=c0s[cell])
                c_prev.append(t_)
            flag_prev = ap_.tile([128, 1], F32, tag="flag")
            nc.sync.dma_start(out=flag_prev[:], in_=flag0[:])
            x_row = ap_.tile([1, 128], MMD, tag="xrow")
            nc.sync.dma_start(out=x_row[:], in_=x0row[:])

            idx_prev = None
            ybuf = None

            for t in range(T):
                # -- 1) L0 gate partials: bias + w_hh0 (run during prev AG_B window)
                g0 = []
                for d in range(2):
                    g = pg.tile([128, 512], F32, tag=f"g0{d}")
                    nc.tensor.matmul(g[:], (ones[:]), (w0a_sb[d][1][:]),
                                     start=True, stop=False)
                    for k in range(8):
                        nc.tensor.matmul(g[:], (h_prev[d][:, k, :]),
                                         (w0_sb[d][:, k, :]),
                                         start=False, stop=False)
                    g0.append(g)
                # -- 3) x transpose (prev step's argmax -> row layout)
                if idx_prev is not None:
                    x_ps = px.tile([1, 128], F32, tag="xps")
                    nc.tensor.transpose(x_ps[:], idx_prev[:], ident[:])
                    x_row = ap_.tile([1, 128], MMD, tag="xrow")
                    nc.vector.tensor_copy(x_row[:], x_ps[:])
                # -- 4) close L0 gates with x contribution
                for d in range(2):
                    nc.tensor.matmul(g0[d][:], (x_row[:]),
                                     (w0a_sb[d][0][:]),
                                     start=False, stop=True)
                # -- 5) L0 elementwise + transpose own slice
                sendA = sp.tile([128, 256], MMD, tag="sendA")
                agA_in = dp.tile([128, 256], MMD, tag="agAi")
                c_new, h_new = [None] * 4, [None] * 4
                for d in range(2):
                    a = gp.tile([128, 512], F32, tag=f"a{d}")
                    nc.scalar.activation(a[:, 0:384], g0[d][:, 0:384], AF.Sigmoid)
                    nc.scalar.activation(a[:, 384:512], g0[d][:, 384:512], AF.Tanh)
                    t1 = ewp.tile([128, 128], F32, tag="t1")
                    nc.vector.tensor_mul(t1[:], a[:, 128:256], c_prev[d][:])
                    t2 = ewp.tile([128, 128], F32, tag="t2")
                    nc.vector.tensor_mul(t2[:], a[:, 0:128], a[:, 384:512])
                    cn = cp.tile([128, 128], F32, tag=f"c{d}")
                    nc.vector.tensor_add(cn[:], t1[:], t2[:])
                    tc2 = ewp.tile([128, 128], F32, tag="tc2")
                    nc.scalar.activation(tc2[:], cn[:], AF.Tanh)
                    h2 = gp.tile([128, 128], F32, tag=f"h2_{d}")
                    nc.vector.tensor_mul(h2[:], a[:, 256:384], tc2[:])
                    c_new[d] = cn
                    ht = pt.tile([128, 128], F32, tag="ht")
                    nc.tensor.transpose(ht[:], h2[:], ident[:])
                    nc.vector.tensor_copy(sendA[:, d * 128:(d + 1) * 128], ht[:])
                    nc.sync.dma_start(out=agA_in[:, d * 128:(d + 1) * 128],
                                      in_=sendA[:, d * 128:(d + 1) * 128])
                # -- 2R) L1 gate partials: bias + w_hh1 (need h1T(t-1))
                g1 = []
                for d in range(2):
                    g = pg.tile([128, 512], F32, tag=f"g1{d}")
                    nc.tensor.matmul(g[:], (ones[:]), (b1_sb[d][:]),
                                     start=True, stop=False)
                    for k in range(8):
                        nc.tensor.matmul(g[:], (h_prev[2 + d][:, k, :]),
                                         (w1h_sb[d][:, k, :]),
                                         start=False, stop=False)
                    g1.append(g)
                # -- 6) AllGather A (h0 slices)
                agA_out = dp.tile([1024, 256], MMD, tag="agAo", addr_space="Shared")
                nc.gpsimd.collective_compute(
                    "AllGather", ALU.bypass, replica_groups=RG,
                    ins=[agA_in.opt()], outs=[agA_out.opt()],
                )
                h0T_new = []
                for d in range(2):
                    t_ = hp.tile([128, 8, 128], MMD, tag=f"h{d}")
                    nc.sync.dma_start(
                        out=t_[:],
                        in_=agA_out[:, d * 128:(d + 1) * 128].rearrange(
                            "(k p) b -> p k b", p=128),
                    )
                    h0T_new.append(t_)
                    h_new[d] = t_
                # -- 7) close L1 gates: w_ih1 over gathered h0 (k-major for overlap)
                for sd in range(2):
                    for k in range(8):
                        for d in range(2):
                            last = sd == 1 and k == 7 and d == 1
                            nc.tensor.matmul(
                                g1[d][:], (h0T_new[sd][:, k, :]),
                                (w1i_sb[d][:, sd * 8 + k, :]),
                                start=False, stop=last,
                            )
                # -- 8) L1 elementwise + transpose + lin partials
                sendBh = sp.tile([128, 256], MMD, tag="sendBh")
                agB_in = dp.tile([128, 320], MMD, tag="agBi")
                sendBl = sp.tile([128, 64], F32, tag="sendBl")
                for d in range(2):
                    a = gp.tile([128, 512], F32, tag=f"a{d}")
                    nc.scalar.activation(a[:, 0:384], g1[d][:, 0:384], AF.Sigmoid)
                    nc.scalar.activation(a[:, 384:512], g1[d][:, 384:512], AF.Tanh)
                    t1 = ewp.tile([128, 128], F32, tag="t1")
                    nc.vector.tensor_mul(t1[:], a[:, 128:256], c_prev[2 + d][:])
                    t2 = ewp.tile([128, 128], F32, tag="t2")
                    nc.vector.tensor_mul(t2[:], a[:, 0:128], a[:, 384:512])
                    cn = cp.tile([128, 128], F32, tag=f"c{2 + d}")
                    nc.vector.tensor_add(cn[:], t1[:], t2[:])
                    tc2 = ewp.tile([128, 128], F32, tag="tc2")
                    nc.scalar.activation(tc2[:], cn[:], AF.Tanh)
                    h2 = gp.tile([128, 128], F32, tag=f"h2_{2 + d}")
                    nc.vector.tensor_mul(h2[:], a[:, 256:384], tc2[:])
                    c_new[2 + d] = cn
                    ht = pt.tile([128, 128], F32, tag="ht")
                    nc.tensor.transpose(ht[:], h2[:], ident[:])
                    nc.vector.tensor_copy(sendBh[:, d * 128:(d + 1) * 128], ht[:])
                    nc.sync.dma_start(out=agB_in[:, d * 128:(d + 1) * 128],
                                      in_=sendBh[:, d * 128:(d + 1) * 128])
                lp = px.tile([128, 64], F32, tag="lp")
                for d in range(2):
                    nc.tensor.matmul(lp[:], (sendBh[:, d * 128:(d + 1) * 128]),
                                     (lin_sb[:, d, :]),
                                     start=(d == 0), stop=(d == 1))
                nc.vector.tensor_copy(sendBl[:], lp[:])
                # -- 9) AllGather B (h1 slices + logit partials)
                nc.sync.dma_start(out=agB_in[:, 256:320].bitcast(F32), in_=sendBl[:])
                agB_out = dp.tile([1024, 320], MMD, tag="agBo", addr_space="Shared")
                nc.gpsimd.collective_compute(
                    "AllGather", ALU.bypass, replica_groups=RG,
                    ins=[agB_in.opt()], outs=[agB_out.opt()],
                )
                LG = gp.tile([128, 8, 64], F32, tag="LG")
                nc.sync.dma_start(
                    out=LG[:],
                    in_=agB_out[:, 256:320].bitcast(F32).rearrange("(c p) v -> p c v", p=128),
                )
                for d in range(2):
                    t_ = hp.tile([128, 8, 128], MMD, tag=f"h{2 + d}")
                    nc.sync.dma_start(
                        out=t_[:],
                        in_=agB_out[:, d * 128:(d + 1) * 128].rearrange(
                            "(k p) b -> p k b", p=128),
                    )
                    h_new[2 + d] = t_
                # -- 10) logits sum + argmax + flag + masked store
                L = ap_.tile([128, 64], F32, tag="L")
                l4 = gp.tile([128, 4, 64], F32, tag="l4")
                nc.vector.tensor_add(l4[:], LG[:, 0:4, :], LG[:, 4:8, :])
                l2 = gp.tile([128, 2, 64], F32, tag="l2")
                nc.vector.tensor_add(l2[:], l4[:, 0:2, :], l4[:, 2:4, :])
                nc.vector.tensor_add(L[:], l2[:, 0, :], l2[:, 1, :])
                nc.vector.tensor_add(L[:], L[:], linb_sb[:])
                m = ap_.tile([128, 1], F32, tag="m")
                nc.vector.tensor_reduce(m[:], L[:], axis=mybir.AxisListType.X,
                                        op=ALU.max)
                ismax = ap_.tile([128, 64], F32, tag="ismax")
                nc.vector.tensor_scalar(ismax[:], L[:], m[:], None, op0=ALU.is_ge)
                cand = ap_.tile([128, 64], F32, tag="cand")
                nc.vector.tensor_mul(cand[:], ismax[:], iot_sb[:])
                nc.vector.tensor_scalar(cand[:], cand[:], 100.0, None, op0=ALU.add)
                idx = ap_.tile([128, 1], F32, tag="idx")
                nc.vector.tensor_reduce(idx[:], cand[:], axis=mybir.AxisListType.X,
                                        op=ALU.min)
                flagb = ap_.tile([128, 1], F32, tag="flagb")
                nc.vector.tensor_scalar(flagb[:], idx[:], 1.0, None, op0=ALU.is_equal)
                fnew = ap_.tile([128, 1], F32, tag="flag")
                nc.vector.tensor_max(fnew[:], flag_prev[:], flagb[:])
                tk = ap_.tile([128, 64], F32, tag="tk")
                nc.vector.tensor_mul(tk[:], L[:], nk_sb[:])
                tk2 = ap_.tile([128, 64], F32, tag="tk2")
                nc.vector.tensor_scalar(tk2[:], tk[:], fnew[:], None, op0=ALU.mult)
                if t % CHUNK == 0:
                    ybuf = yp.tile([128, CHUNK, 64], F32, tag="ybuf")
                nc.vector.tensor_sub(ybuf[:, t % CHUNK, :], L[:], tk2[:])
                if t % CHUNK == CHUNK - 1:
                    nc.sync.dma_start(out=y[:, t - CHUNK + 1:t + 1, :], in_=ybuf[:])
                # carry
                h_prev = h_new
                c_prev = c_new
                flag_prev = fnew
                idx_prev = idx
            if T % CHUNK != 0:
                nfin = T % CHUNK
                nc.sync.dma_start(out=y[:, T - nfin:T, :], in_=ybuf[:, 0:nfin, :])
            for cell in range(4):
                nc.sync.dma_start(out=hT_f[cell], in_=h_prev[cell][:])
                nc.sync.dma_start(out=c_f[cell], in_=c_prev[cell][:])
            nc.sync.dma_start(out=flag_f[:], in_=flag_prev[:])
            nc.sync.dma_start(out=idx_f[:], in_=idx_prev[:])
    nc.finalize()
    return nc


def prep_inputs(h0, c0, w_ih0, w_hh0, b0, w_ih1, w_hh1, b1, lin_w, lin_b):
    """Host-side packing: per-core sliced/transposed weight + state arrays."""
    h0 = np.asarray(h0, np.float32).reshape(2, 2, B, H)
    c0 = np.asarray(c0, np.float32).reshape(2, 2, B, H)
    w_ih0 = np.asarray(w_ih0, np.float32)
    w_hh0 = np.asarray(w_hh0, np.float32)
    b0 = np.asarray(b0, np.float32)
    w_ih1 = np.asarray(w_ih1, np.float32)
    w_hh1 = np.asarray(w_hh1, np.float32)
    b1 = np.asarray(b1, np.float32)
    lin_w = np.asarray(lin_w, np.float32)
    lin_b = np.asarray(lin_b, np.float32)

    iota = np.broadcast_to((np.arange(V) - 100.0).astype(np.float32), (128, V)).copy()
    linbb = np.broadcast_to(lin_b, (128, V)).copy()
    nk = np.ones((128, V), np.float32)
    nk[:, KEEP_IDX] = 0.0

    hT0 = np.zeros((4, 128, 8, B), np.float32)
    for l in range(2):
        for d in range(2):
            cell = l * 2 + d
            hT0[cell] = h0[l, d].T.reshape(8, 128, B).transpose(1, 0, 2)

    in_maps = []
    for c in range(NCORES):
        rows = np.concatenate([np.arange(gb + c * 128, gb + c * 128 + 128)
                               for gb in GBASE])

        def packT(w, kt):
            # w: (4H, K*128) -> select rows -> [p, k, n]
            sel = w[rows, :]  # (512, kt*128)
            return np.ascontiguousarray(
                sel.reshape(512, kt, 128).transpose(2, 1, 0))

        w0T = np.stack([packT(w_hh0[d], 8) for d in range(2)])
        w1iT = np.stack([packT(w_ih1[d], 16) for d in range(2)])
        w1hT = np.stack([packT(w_hh1[d], 8) for d in range(2)])
        w0aug = np.stack([np.stack([w_ih0[d][rows, 0], b0[d][rows]])
                          for d in range(2)])
        b1row = np.stack([b1[d][rows][None, :] for d in range(2)])
        linTc = np.stack(
            [lin_w[:, c * 128:(c + 1) * 128].T,
             lin_w[:, H + c * 128:H + (c + 1) * 128].T], axis=1)
        c0slice = np.zeros((4, 128, 128), np.float32)
        for l in range(2):
            for d in range(2):
                c0slice[l * 2 + d] = c0[l, d][:, c * 128:(c + 1) * 128]
        in_maps.append({
            "w0T": tf32_round(np.ascontiguousarray(w0T)),
            "w0aug": tf32_round(np.ascontiguousarray(w0aug)),
            "w1iT": tf32_round(np.ascontiguousarray(w1iT)),
            "w1hT": tf32_round(np.ascontiguousarray(w1hT)),
            "b1row": tf32_round(np.ascontiguousarray(b1row)),
            "linTc": tf32_round(np.ascontiguousarray(linTc)),
            "linb": linbb,
            "iotam": iota,
            "notkeep": nk,
            "hT0": tf32_round(hT0),
            "c0s": np.ascontiguousarray(c0slice),
            "onesrow": np.ones((1, 128), np.float32),
            "x0row": np.full((1, 128), MASK_IDX, np.float32),
            "flag0": np.zeros((128, 1), np.float32),
        })
    return in_maps


_NC_CACHE = {}


def _get_nc(T):
    if T not in _NC_CACHE:
        _NC_CACHE[T] = build(T)
    return _NC_CACHE[T]


T_LAUNCH = 256


def kernel(h0, c0, w_ih0, w_hh0, b0, w_ih1, w_hh1, b1, lin_w, lin_b,
           decoder_output_length, batch_size, _want_results=False):
    T = int(decoder_output_length)
    assert int(batch_size) == B
    in_maps = prep_inputs(h0, c0, w_ih0, w_hh0, b0, w_ih1, w_hh1, b1,
                          lin_w, lin_b)
    chunks = []
    t_done = 0
    res = None
    while t_done < T:
        t_this = min(T_LAUNCH, T - t_done)
        nc = _get_nc(t_this)
        res = bass_utils.run_bass_kernel_spmd(nc, in_maps,
                                              core_ids=list(range(NCORES)))
        chunks.append(res.results[0]["y"])
        t_done += t_this
        if t_done < T:
            idxs = res.results[0]["idx_f"]  # (128,1) float indices
            xrow = np.ascontiguousarray(idxs.reshape(1, 128))
            for c in range(NCORES):
                rc = res.results[c]
                in_maps[c] = dict(in_maps[c])
                in_maps[c]["hT0"] = rc["hT_f"]
                in_maps[c]["c0s"] = rc["c_f"]
                in_maps[c]["flag0"] = rc["flag_f"]
                in_maps[c]["x0row"] = xrow
    out = np.concatenate(chunks, axis=1) if len(chunks) > 1 else chunks[0]
    if _want_results:
        return out, res
    return out



# revision 22
# speedup vs baseline: 2.2008x; 2.2008x over previous
"""Trainium2 Bass kernel for nn_Decoder (2-layer bidirectional LSTM decoder,
autoregressive argmax feedback, T=512 steps, B=128, H=1024, V=64).

Strategy: 8-way tensor parallel over the 4H gate dimension. Each core holds a
512-wide slice of every gate projection (re-ordered [i,f,o,g] so activations
fuse), keeps the LSTM recurrence state resident, and exchanges state via three
collectives per step arranged so that almost all compute hides inside
collective windows:
  AGH0  AllGather of h0(t) slices   -> feeds the L1 input matmuls
  AR    AllReduce  of logit partials -> feeds the argmax / x feedback
  AGH1  AllGather of h1(t) slices   -> feeds next step's L1 h-recurrence
The argmax -> x -> L0 chain runs inside the AGH1 window, the L0/L1
h-recurrence partial matmuls run inside the AR / AGH0 windows, and only the
L1 input-projection matmuls (which need the gathered h0) remain exposed.
Their DMA loads are split so the matmul groups become runnable progressively,
which keeps the tensor engine p-state ramped.
"""

import os
import sys

import numpy as np

sys.path.insert(0, "/opt/trn_rl_repo")

import concourse.bass as bass  # noqa: E402
import concourse.mybir as mybir  # noqa: E402
import concourse.tile as tile  # noqa: E402
from concourse import bacc  # noqa: E402
from concourse import bass_utils  # noqa: E402
from concourse.masks import make_identity  # noqa: E402

H = 1024
V = 64
B = 128
NCORES = 8
MASK_IDX = 4.0
KEEP_IDX = 3
T_STEPS = int(os.environ.get("DEC_T", "512"))
CHUNK = int(os.environ.get("DEC_CHUNK", "8"))
MM_DT = mybir.dt.float32r if os.environ.get("DEC_MMDT", "fp32r") == "fp32r" else mybir.dt.float32
F32 = mybir.dt.float32
MMD = MM_DT
AF = mybir.ActivationFunctionType
ALU = mybir.AluOpType

# gate blocks packed per-core as [i, f, o, g] (torch order in rows is i,f,g,o)
GBASE = [0, H, 3 * H, 2 * H]

# h0T gathered-load split: chunk groups per input direction (pacing)
LOAD_GROUPS = [(0, 1), (1, 4), (4, 8)]


def tf32_round(x):
    if MM_DT == F32:
        return np.asarray(x, np.float32)
    xi = np.asarray(x, np.float32).view(np.uint32)
    xi = (xi + np.uint32(1 << 12)) & np.uint32(0xFFFFE000)
    return xi.view(np.float32)


def build(T=T_STEPS):
    nc = bacc.Bacc("TRN2", num_devices=NCORES)
    RG = [list(range(NCORES))]

    din = dict(kind="ExternalInput")
    w0T = nc.dram_tensor("w0T", [2, 128, 8, 512], MMD, **din)
    w0aug = nc.dram_tensor("w0aug", [2, 2, 512], MMD, **din)
    w1iT = nc.dram_tensor("w1iT", [2, 128, 16, 512], MMD, **din)
    w1hT = nc.dram_tensor("w1hT", [2, 128, 8, 512], MMD, **din)
    b1row = nc.dram_tensor("b1row", [2, 1, 512], MMD, **din)
    linTc = nc.dram_tensor("linTc", [128, 2, 64], MMD, **din)
    linrow = nc.dram_tensor("linrow", [1, 64], MMD, **din)
    iotam = nc.dram_tensor("iotam", [128, 64], F32, **din)
    notkeep = nc.dram_tensor("notkeep", [128, 64], F32, **din)
    hT0 = nc.dram_tensor("hT0", [4, 128, 8, 128], MMD, **din)
    c0s = nc.dram_tensor("c0s", [4, 128, 128], F32, **din)
    onesrow = nc.dram_tensor("onesrow", [1, 128], MMD, **din)
    x0row = nc.dram_tensor("x0row", [1, 128], MMD, **din)
    flag0 = nc.dram_tensor("flag0", [128, 1], F32, **din)
    hT_f = nc.dram_tensor("hT_f", [4, 128, 8, 128], MMD, kind="ExternalOutput")
    c_f = nc.dram_tensor("c_f", [4, 128, 128], F32, kind="ExternalOutput")
    flag_f = nc.dram_tensor("flag_f", [128, 1], F32, kind="ExternalOutput")
    idx_f = nc.dram_tensor("idx_f", [128, 1], F32, kind="ExternalOutput")
    y = nc.dram_tensor("y", [B, T, V], F32, kind="ExternalOutput")

    with tile.TileContext(nc) as tc:
        import contextlib

        ctx = contextlib.ExitStack()
        with ctx:
            wp = ctx.enter_context(tc.tile_pool(name="weights", bufs=1))
            hp = ctx.enter_context(tc.tile_pool(name="hstate", bufs=2))
            cp = ctx.enter_context(tc.tile_pool(name="cstate", bufs=2))
            gp = ctx.enter_context(tc.tile_pool(name="gact", bufs=1))
            ewp = ctx.enter_context(tc.tile_pool(name="ew", bufs=1))
            sp = ctx.enter_context(tc.tile_pool(name="send", bufs=2))
            ap_ = ctx.enter_context(tc.tile_pool(name="amax", bufs=2))
            yp = ctx.enter_context(tc.tile_pool(name="ybuf", bufs=1))
            pg = ctx.enter_context(tc.tile_pool(name="pgates", bufs=1, space="PSUM"))
            pt = ctx.enter_context(tc.tile_pool(name="ptrans", bufs=2, space="PSUM"))
            px = ctx.enter_context(tc.tile_pool(name="pmisc", bufs=1, space="PSUM"))
            dp = ctx.enter_context(tc.tile_pool(name="dram", bufs=2, space="DRAM"))

            # ---- load weights + constants (once) ----
            w0_sb, w0a_sb, w1i_sb, w1h_sb, b1_sb = [], [], [], [], []
            for d in range(2):
                t_ = wp.tile([128, 8, 512], MMD, tag=f"w0_{d}")
                nc.sync.dma_start(out=t_[:], in_=w0T[d])
                w0_sb.append(t_)
                tb = wp.tile([1, 512], MMD, tag=f"w0b_{d}")
                nc.sync.dma_start(out=tb[:], in_=w0aug[d, 1:2])
                tx = wp.tile([1, 512], MMD, tag=f"w0x_{d}")
                nc.sync.dma_start(out=tx[:], in_=w0aug[d, 0:1])
                w0a_sb.append((tx, tb))
                t_ = wp.tile([128, 16, 512], MMD, tag=f"w1i_{d}")
                nc.sync.dma_start(out=t_[:], in_=w1iT[d])
                w1i_sb.append(t_)
                t_ = wp.tile([128, 8, 512], MMD, tag=f"w1h_{d}")
                nc.sync.dma_start(out=t_[:], in_=w1hT[d])
                w1h_sb.append(t_)
                t_ = wp.tile([1, 512], MMD, tag=f"b1_{d}")
                nc.sync.dma_start(out=t_[:], in_=b1row[d])
                b1_sb.append(t_)
            lin_sb = wp.tile([128, 2, 64], MMD, tag="lin")
            nc.sync.dma_start(out=lin_sb[:], in_=linTc[:])
            linr_sb = wp.tile([1, 64], MMD, tag="linr")
            nc.sync.dma_start(out=linr_sb[:], in_=linrow[:])
            iot_sb = wp.tile([128, 64], F32, tag="iot")
            nc.sync.dma_start(out=iot_sb[:], in_=iotam[:])
            nk_sb = wp.tile([128, 64], F32, tag="nk")
            nc.sync.dma_start(out=nk_sb[:], in_=notkeep[:])
            ident = wp.tile([128, 128], F32, tag="ident")
            make_identity(nc, ident[:])
            ones = wp.tile([1, 128], MMD, tag="ones")
            nc.sync.dma_start(out=ones[:], in_=onesrow[:])

            # ---- initial state ----
            h_prev = []
            for cell in range(4):
                t_ = hp.tile([128, 8, 128], MMD, tag=f"h{cell}")
                nc.sync.dma_start(out=t_[:], in_=hT0[cell])
                h_prev.append(t_)
            c_prev = []
            for cell in range(4):
                t_ = cp.tile([128, 128], F32, tag=f"c{cell}")
                nc.sync.dma_start(out=t_[:], in_=c0s[cell])
                c_prev.append(t_)
            flag_prev = ap_.tile([128, 1], F32, tag="flag")
            nc.sync.dma_start(out=flag_prev[:], in_=flag0[:])
            x_row = ap_.tile([1, 128], MMD, tag="xrow")
            nc.sync.dma_start(out=x_row[:], in_=x0row[:])

            def lstm_ew_pre(g, c_in, cell):
                """gate PSUM [128,512] (i,f,o,g blocks) + c_in -> (cn, h2),
                Act/DVE only (no PE ops)."""
                a = gp.tile([128, 512], F32, tag=f"a{cell}")
                nc.scalar.activation(a[:, 0:384], g[:, 0:384], AF.Sigmoid)
                nc.scalar.activation(a[:, 384:512], g[:, 384:512], AF.Tanh)
                t1 = ewp.tile([128, 128], F32, tag=f"t1_{cell}")
                nc.vector.tensor_mul(t1[:], a[:, 128:256], c_in[:])
                t2 = ewp.tile([128, 128], F32, tag=f"t2_{cell}")
                nc.vector.tensor_mul(t2[:], a[:, 0:128], a[:, 384:512])
                cn = cp.tile([128, 128], F32, tag=f"c{cell}")
                nc.vector.tensor_add(cn[:], t1[:], t2[:])
                tc2 = ewp.tile([128, 128], F32, tag=f"tc2_{cell}")
                nc.scalar.activation(tc2[:], cn[:], AF.Tanh)
                h2 = gp.tile([128, 128], F32, tag=f"h2_{cell}")
                nc.vector.tensor_mul(h2[:], a[:, 256:384], tc2[:])
                return cn, h2

            def h_transpose(h2, dst, dst_col):
                ht = pt.tile([128, 128], F32, tag="ht")
                nc.tensor.transpose(ht[:], h2[:], ident[:])
                nc.vector.tensor_copy(dst[:, dst_col:dst_col + 128], ht[:])

            def lstm_ew(g, c_in, cell, dst, dst_col):
                cn, h2 = lstm_ew_pre(g, c_in, cell)
                h_transpose(h2, dst, dst_col)
                return cn

            # ---- prologue: L0(0) + stage + AGH0(0) ----
            g0 = []
            for d in range(2):
                g = pg.tile([128, 512], F32, tag=f"g0{d}")
                nc.tensor.matmul(g[:], (ones[:]), (w0a_sb[d][1][:]),
                                 start=True, stop=False)
                for k in range(8):
                    nc.tensor.matmul(g[:], (h_prev[d][:, k, :]),
                                     (w0_sb[d][:, k, :]),
                                     start=False, stop=False)
                nc.tensor.matmul(g[:], (x_row[:]), (w0a_sb[d][0][:]),
                                 start=False, stop=True)
                g0.append(g)
            sendA = sp.tile([128, 256], MMD, tag="sendA")
            agA_in = dp.tile([128, 256], MMD, tag="agAi")
            c_new = [None] * 4
            for d in range(2):
                c_new[d] = lstm_ew(g0[d], c_prev[d], d, sendA, d * 128)
                nc.sync.dma_start(out=agA_in[:, d * 128:(d + 1) * 128],
                                  in_=sendA[:, d * 128:(d + 1) * 128])
            c_prev = [c_new[0], c_new[1], c_prev[2], c_prev[3]]
            agA_out = dp.tile([1024, 256], MMD, tag="agAo", addr_space="Shared")
            nc.gpsimd.collective_compute(
                "AllGather", ALU.bypass, replica_groups=RG,
                ins=[agA_in.opt()], outs=[agA_out.opt()],
            )

            idx = None
            ybuf = None
            agB_outs = [None, None]

            for t in range(T):
                last = t == T - 1
                # -- 1) h1T-d0(t-1) gathered load (skip t=0: prologue loaded)
                if t > 0:
                    t_ = hp.tile([128, 8, 128], MMD, tag="h2")
                    nc.sync.dma_start(
                        out=t_[:],
                        in_=agB_outs[0][:, 0:128].rearrange(
                            "(k p) b -> p k b", p=128),
                    )
                    h_prev[2] = t_
                # -- 2) g1(t) partials: bias both dirs + w_hh1 dir0
                #       (hh-d0 hides in the AGH0(t) window)
                g1 = []
                for d in range(2):
                    g = pg.tile([128, 512], F32, tag=f"g1{d}")
                    nc.tensor.matmul(g[:], (ones[:]), (b1_sb[d][:]),
                                     start=True, stop=False)
                    g1.append(g)
                for k in range(8):
                    nc.tensor.matmul(g1[0][:], (h_prev[2][:, k, :]),
                                     (w1h_sb[0][:, k, :]),
                                     start=False, stop=False)
                # -- 3) h0T(t) gathered loads, split for progressive pacing
                h0n = []
                for d in range(2):
                    t_ = hp.tile([128, 8, 128], MMD, tag=f"h{d}")
                    for k0, k1 in LOAD_GROUPS:
                        nc.sync.dma_start(
                            out=t_[:, k0:k1, :],
                            in_=agA_out[k0 * 128:k1 * 128,
                                        d * 128:(d + 1) * 128].rearrange(
                                "(k p) b -> p k b", p=128),
                        )
                    h0n.append(t_)
                    h_prev[d] = t_
                # -- 4) ih(t) for both output dirs (hides in AGH1b(t-1) win)
                for d in range(2):
                    for sd in range(2):
                        for k0, k1 in LOAD_GROUPS:
                            for k in range(k0, k1):
                                lastmm = d == 0 and sd == 1 and k == 7
                                nc.tensor.matmul(
                                    g1[d][:], (h0n[sd][:, k, :]),
                                    (w1i_sb[d][:, sd * 8 + k, :]),
                                    start=False, stop=lastmm,
                                )
                # -- 5a) ew-d0 + transpose + logit partial half 0 + AR0: all
                #        hide in the AGH1b(t-1) window (ready after ih stop)
                sendB = sp.tile([128, 256], MMD, tag="sendB")
                lpt = px.tile([128, 128], F32, tag="lp")
                lp0 = lpt[:, 0:64]
                lp1 = lpt[:, 64:128]
                nc.tensor.matmul(lp0[:], (ones[:]), (linr_sb[:]),
                                 start=True, stop=False)
                c_new[2], h2_0 = lstm_ew_pre(g1[0], c_prev[2], 2)
                h_transpose(h2_0, sendB, 0)
                nc.tensor.matmul(lp0[:], (sendB[:, 0:128]), (lin_sb[:, 0, :]),
                                 start=False, stop=True)
                agB0_in = dp.tile([128, 129], MMD, tag="agB0i")
                nc.sync.dma_start(out=agB0_in[:, 0:128], in_=sendB[:, 0:128])
                sendBl0 = sp.tile([128, 64], F32, tag="sendBl0")
                nc.vector.tensor_copy(sendBl0[:], lp0[:])
                ar0_in = dp.tile([128, 64], F32, tag="ar0i")
                nc.sync.dma_start(out=ar0_in[:], in_=sendBl0[:])
                ar0_out = dp.tile([128, 64], F32, tag="ar0o",
                                  addr_space="Shared")
                nc.gpsimd.collective_compute(
                    "AllReduce", ALU.add, replica_groups=RG,
                    ins=[ar0_in.opt()], outs=[ar0_out.opt()],
                )
                # -- 4b) h1T-d1(t-1) load (paced) + hh-d1(t) + ew-d1 + AR1:
                #        this tail hides in the AR0(t) window
                if t > 0:
                    t_ = hp.tile([128, 8, 128], MMD, tag="h3")
                    for k0, k1 in LOAD_GROUPS:
                        nc.sync.dma_start(
                            out=t_[:, k0:k1, :],
                            in_=agB_outs[1][k0 * 128:k1 * 128, 0:128].rearrange(
                                "(k p) b -> p k b", p=128),
                        )
                    h_prev[3] = t_
                for k0, k1 in LOAD_GROUPS:
                    for k in range(k0, k1):
                        nc.tensor.matmul(g1[1][:], (h_prev[3][:, k, :]),
                                         (w1h_sb[1][:, k, :]),
                                         start=False, stop=(k == 7))
                c_new[3], h2_1 = lstm_ew_pre(g1[1], c_prev[3], 3)
                h_transpose(h2_1, sendB, 128)
                nc.tensor.matmul(lp1[:], (sendB[:, 128:256]), (lin_sb[:, 1, :]),
                                 start=True, stop=True)
                c_prev = [c_prev[0], c_prev[1], c_new[2], c_new[3]]
                sendBl1 = sp.tile([128, 64], F32, tag="sendBl1")
                nc.vector.tensor_copy(sendBl1[:], lp1[:])
                ar1_in = dp.tile([128, 64], F32, tag="ar1i")
                nc.sync.dma_start(out=ar1_in[:], in_=sendBl1[:])
                agB1_in = dp.tile([128, 129], MMD, tag="agB1i")
                nc.sync.dma_start(out=agB1_in[:, 0:128], in_=sendB[:, 128:256])
                # -- 7) AR1(t) + AGH1a(t). Device order forced by dep DMAs:
                # AR1 before AGH1a (d2d col from ar1_in), AGH1b after
                # AGH0(t+1) (d2d col from agA_in).
                ar1_out = dp.tile([128, 64], F32, tag="ar1o",
                                  addr_space="Shared")
                nc.gpsimd.collective_compute(
                    "AllReduce", ALU.add, replica_groups=RG,
                    ins=[ar1_in.opt()], outs=[ar1_out.opt()],
                )
                nc.sync.dma_start(out=agB0_in[:, 128:129].bitcast(F32),
                                  in_=ar1_in[:, 0:1])
                agB0_out = dp.tile([1024, 129], MMD, tag="agB0o",
                                   addr_space="Shared")
                nc.gpsimd.collective_compute(
                    "AllGather", ALU.bypass, replica_groups=RG,
                    ins=[agB0_in.opt()], outs=[agB0_out.opt()],
                )
                agB_outs = [agB0_out, None]
                # -- 8) g0(t+1) partials: bias + w_hh0 (hidden in AR window)
                if not last:
                    g0 = []
                    for d in range(2):
                        g = pg.tile([128, 512], F32, tag=f"g0{d}")
                        nc.tensor.matmul(g[:], (ones[:]), (w0a_sb[d][1][:]),
                                         start=True, stop=False)
                        for k in range(8):
                            nc.tensor.matmul(g[:], (h0n[d][:, k, :]),
                                             (w0_sb[d][:, k, :]),
                                             start=False, stop=False)
                        g0.append(g)
                # -- 9) logits load (both AR halves) + argmax chain
                L0h = ap_.tile([128, 64], F32, tag="L0h")
                nc.sync.dma_start(out=L0h[:], in_=ar0_out[:])
                L1h = ap_.tile([128, 64], F32, tag="L1h")
                nc.sync.dma_start(out=L1h[:], in_=ar1_out[:])
                L = ap_.tile([128, 64], F32, tag="L")
                nc.vector.tensor_add(L[:], L0h[:], L1h[:])
                m = ap_.tile([128, 1], F32, tag="m")
                nc.vector.tensor_reduce(m[:], L[:], axis=mybir.AxisListType.X,
                                        op=ALU.max)
                ismax = ap_.tile([128, 64], F32, tag="ismax")
                nc.vector.tensor_scalar(ismax[:], L[:], m[:], None,
                                        op0=ALU.is_ge)
                cand = ap_.tile([128, 64], F32, tag="cand")
                nc.vector.tensor_mul(cand[:], ismax[:], iot_sb[:])
                idxm = ap_.tile([128, 1], F32, tag="idxm")
                nc.vector.tensor_reduce(idxm[:], cand[:],
                                        axis=mybir.AxisListType.X, op=ALU.min)
                idx = ap_.tile([128, 1], F32, tag="idx")
                nc.vector.tensor_scalar(idx[:], idxm[:], 100.0, None,
                                        op0=ALU.add)
                # -- 10) x feedback + close g0(t+1) (hidden in AGH1 window)
                if not last:
                    x_ps = px.tile([1, 128], F32, tag="xps")
                    nc.tensor.transpose(x_ps[:], idx[:], ident[:])
                    x_row = ap_.tile([1, 128], MMD, tag="xrow")
                    nc.vector.tensor_copy(x_row[:], x_ps[:])
                    for d in range(2):
                        nc.tensor.matmul(g0[d][:], (x_row[:]),
                                         (w0a_sb[d][0][:]),
                                         start=False, stop=True)
                # -- 11) flag + masked store
                flagb = ap_.tile([128, 1], F32, tag="flagb")
                nc.vector.tensor_scalar(flagb[:], idx[:], 1.0, None,
                                        op0=ALU.is_equal)
                fnew = ap_.tile([128, 1], F32, tag="flag")
                nc.vector.tensor_max(fnew[:], flag_prev[:], flagb[:])
                tk = ap_.tile([128, 64], F32, tag="tk")
                nc.vector.tensor_mul(tk[:], L[:], nk_sb[:])
                tk2 = ap_.tile([128, 64], F32, tag="tk2")
                nc.vector.tensor_scalar(tk2[:], tk[:], fnew[:], None,
                                        op0=ALU.mult)
                if t % CHUNK == 0:
                    ybuf = yp.tile([128, CHUNK, 64], F32, tag="ybuf")
                nc.vector.tensor_sub(ybuf[:, t % CHUNK, :], L[:], tk2[:])
                if t % CHUNK == CHUNK - 1:
                    nc.sync.dma_start(out=y[:, t - CHUNK + 1:t + 1, :],
                                      in_=ybuf[:])
                flag_prev = fnew
                # -- 12) L0(t+1) elementwise + stage + AGH0(t+1) + AGH1b(t)
                if not last:
                    sendA = sp.tile([128, 256], MMD, tag="sendA")
                    agA_in = dp.tile([128, 256], MMD, tag="agAi")
                    for d in range(2):
                        c_new[d] = lstm_ew(g0[d], c_prev[d], d, sendA, d * 128)
                    nc.sync.dma_start(out=agA_in[:], in_=sendA[:])
                    c_prev = [c_new[0], c_new[1], c_prev[2], c_prev[3]]
                    agA_out = dp.tile([1024, 256], MMD, tag="agAo",
                                      addr_space="Shared")
                    nc.gpsimd.collective_compute(
                        "AllGather", ALU.bypass, replica_groups=RG,
                        ins=[agA_in.opt()], outs=[agA_out.opt()],
                    )
                    # dep col: AGH1b(t) becomes ready only after agA(t+1)
                    nc.sync.dma_start(out=agB1_in[:, 128:129],
                                      in_=agA_in[:, 0:1])
                else:
                    nc.sync.dma_start(out=agB1_in[:, 128:129].bitcast(F32),
                                      in_=ar1_in[:, 0:1])
                agB1_out = dp.tile([1024, 129], MMD, tag="agB1o",
                                   addr_space="Shared")
                nc.gpsimd.collective_compute(
                    "AllGather", ALU.bypass, replica_groups=RG,
                    ins=[agB1_in.opt()], outs=[agB1_out.opt()],
                )
                agB_outs = [agB0_out, agB1_out]
            # ---- epilogue ----
            if T % CHUNK != 0:
                nfin = T % CHUNK
                nc.sync.dma_start(out=y[:, T - nfin:T, :], in_=ybuf[:, 0:nfin, :])
            for d in range(2):
                t_ = hp.tile([128, 8, 128], MMD, tag=f"h{2 + d}")
                nc.sync.dma_start(
                    out=t_[:],
                    in_=agB_outs[d][:, 0:128].rearrange(
                        "(k p) b -> p k b", p=128),
                )
                h_prev[2 + d] = t_
            for cell in range(4):
                nc.sync.dma_start(out=hT_f[cell], in_=h_prev[cell][:])
                nc.sync.dma_start(out=c_f[cell], in_=c_prev[cell][:])
            nc.sync.dma_start(out=flag_f[:], in_=flag_prev[:])
            nc.sync.dma_start(out=idx_f[:], in_=idx[:])
    nc.finalize()
    return nc


def prep_inputs(h0, c0, w_ih0, w_hh0, b0, w_ih1, w_hh1, b1, lin_w, lin_b):
    """Host-side packing: per-core sliced/transposed weight + state arrays."""
    h0 = np.asarray(h0, np.float32).reshape(2, 2, B, H)
    c0 = np.asarray(c0, np.float32).reshape(2, 2, B, H)
    w_ih0 = np.asarray(w_ih0, np.float32)
    w_hh0 = np.asarray(w_hh0, np.float32)
    b0 = np.asarray(b0, np.float32)
    w_ih1 = np.asarray(w_ih1, np.float32)
    w_hh1 = np.asarray(w_hh1, np.float32)
    b1 = np.asarray(b1, np.float32)
    lin_w = np.asarray(lin_w, np.float32)
    lin_b = np.asarray(lin_b, np.float32)

    iota = np.broadcast_to((np.arange(V) - 100.0).astype(np.float32),
                           (128, V)).copy()
    nk = np.ones((128, V), np.float32)
    nk[:, KEEP_IDX] = 0.0

    hT0 = np.zeros((4, 128, 8, B), np.float32)
    for l in range(2):
        for d in range(2):
            cell = l * 2 + d
            hT0[cell] = h0[l, d].T.reshape(8, 128, B).transpose(1, 0, 2)

    in_maps = []
    for c in range(NCORES):
        rows = np.concatenate([np.arange(gb + c * 128, gb + c * 128 + 128)
                               for gb in GBASE])

        def packT(w, kt):
            # w: (4H, K*128) -> select rows -> [p, k, n]
            sel = w[rows, :]  # (512, kt*128)
            return np.ascontiguousarray(
                sel.reshape(512, kt, 128).transpose(2, 1, 0))

        w0T = np.stack([packT(w_hh0[d], 8) for d in range(2)])
        w1iT = np.stack([packT(w_ih1[d], 16) for d in range(2)])
        w1hT = np.stack([packT(w_hh1[d], 8) for d in range(2)])
        w0aug = np.stack([np.stack([w_ih0[d][rows, 0], b0[d][rows]])
                          for d in range(2)])
        b1row = np.stack([b1[d][rows][None, :] for d in range(2)])
        linTc = np.stack(
            [lin_w[:, c * 128:(c + 1) * 128].T,
             lin_w[:, H + c * 128:H + (c + 1) * 128].T], axis=1)
        c0slice = np.zeros((4, 128, 128), np.float32)
        for l in range(2):
            for d in range(2):
                c0slice[l * 2 + d] = c0[l, d][:, c * 128:(c + 1) * 128]
        in_maps.append({
            "w0T": tf32_round(np.ascontiguousarray(w0T)),
            "w0aug": tf32_round(np.ascontiguousarray(w0aug)),
            "w1iT": tf32_round(np.ascontiguousarray(w1iT)),
            "w1hT": tf32_round(np.ascontiguousarray(w1hT)),
            "b1row": tf32_round(np.ascontiguousarray(b1row)),
            "linTc": tf32_round(np.ascontiguousarray(linTc)),
            "linrow": tf32_round((lin_b / NCORES)[None, :]),
            "iotam": iota,
            "notkeep": nk,
            "hT0": tf32_round(hT0),
            "c0s": np.ascontiguousarray(c0slice),
            "onesrow": np.ones((1, 128), np.float32),
            "x0row": np.full((1, 128), MASK_IDX, np.float32),
            "flag0": np.zeros((128, 1), np.float32),
        })
    return in_maps


_NC_CACHE = {}


def _get_nc(T):
    if T not in _NC_CACHE:
        _NC_CACHE[T] = build(T)
    return _NC_CACHE[T]


T_LAUNCH = 256


def kernel(h0, c0, w_ih0, w_hh0, b0, w_ih1, w_hh1, b1, lin_w, lin_b,
           decoder_output_length, batch_size, _want_results=False):
    T = int(decoder_output_length)
    assert int(batch_size) == B
    in_maps = prep_inputs(h0, c0, w_ih0, w_hh0, b0, w_ih1, w_hh1, b1,
                          lin_w, lin_b)
    chunks = []
    t_done = 0
    res = None
    while t_done < T:
        t_this = min(T_LAUNCH, T - t_done)
        nc = _get_nc(t_this)
        res = bass_utils.run_bass_kernel_spmd(nc, in_maps,
                                              core_ids=list(range(NCORES)))
        chunks.append(res.results[0]["y"])
        t_done += t_this
        if t_done < T:
            idxs = res.results[0]["idx_f"]  # (128,1) float indices
            xrow = np.ascontiguousarray(idxs.reshape(1, 128))
            for c in range(NCORES):
                rc = res.results[c]
                in_maps[c] = dict(in_maps[c])
                in_maps[c]["hT0"] = rc["hT_f"]
                in_maps[c]["c0s"] = rc["c_f"]
                in_maps[c]["flag0"] = rc["flag_f"]
                in_maps[c]["x0row"] = xrow
    out = np.concatenate(chunks, axis=1) if len(chunks) > 1 else chunks[0]
    if _want_results:
        return out, res
    return out


# revision 24
# speedup vs baseline: 2.2123x; 1.0052x over previous
"""Trainium2 Bass kernel for nn_Decoder (2-layer bidirectional LSTM decoder,
autoregressive argmax feedback, T=512 steps, B=128, H=1024, V=64).

Strategy: 8-way tensor parallel over the 4H gate dimension. Each core holds a
512-wide slice of every gate projection (re-ordered [i,f,o,g] so activations
fuse), keeps the LSTM recurrence state resident, and exchanges state via five
collectives per step, scheduled back-to-back so that every piece of compute
hides inside some collective's window (collectives run on dedicated cores
concurrently with the engines):
  AR0    AllReduce of logit partial half 0 (lin_b/8 + dir-0 h1 term)
         -> its window covers the hh-d1 matmuls + dir-1 elementwise tail
  AR1    AllReduce of logit partial half 1 (dir-1 h1 term)
         -> its window covers next step's L0 h-recurrence partials
  AGH1a  AllGather of h1T dir-0 slices
         -> its window covers argmax -> x -> L0 close/elementwise/staging
  AGH0   AllGather of h0T(t+1) slices (both dirs)
         -> its window covers the hh-d0 matmuls
  AGH1b  AllGather of h1T dir-1 slices
         -> its window covers the 32 L1 input-projection matmuls + ew-d0
Logits are AR0out + AR1out. Device order is forced where the readiness-based
tile scheduler would invert it, via tiny DRAM->DRAM dependency-column DMAs
(AGH1a only becomes ready after AR1's input is staged; AGH1b only after
AGH0(t+1)'s input is staged). The bidirectional split works because each
LSTM direction's h-recurrence contracts only its own direction's h state.
Gathered-state DMA loads are split (igniter chunk first) so dependent matmul
groups become runnable progressively, keeping the PE p-state ramped.
"""

import os
import sys

import numpy as np

sys.path.insert(0, "/opt/trn_rl_repo")

import concourse.bass as bass  # noqa: E402
import concourse.mybir as mybir  # noqa: E402
import concourse.tile as tile  # noqa: E402
from concourse import bacc  # noqa: E402
from concourse import bass_utils  # noqa: E402
from concourse.masks import make_identity  # noqa: E402

H = 1024
V = 64
B = 128
NCORES = 8
MASK_IDX = 4.0
KEEP_IDX = 3
T_STEPS = int(os.environ.get("DEC_T", "512"))
CHUNK = int(os.environ.get("DEC_CHUNK", "8"))
MM_DT = mybir.dt.float32r if os.environ.get("DEC_MMDT", "fp32r") == "fp32r" else mybir.dt.float32
F32 = mybir.dt.float32
MMD = MM_DT
AF = mybir.ActivationFunctionType
ALU = mybir.AluOpType

# gate blocks packed per-core as [i, f, o, g] (torch order in rows is i,f,g,o)
GBASE = [0, H, 3 * H, 2 * H]

# h0T gathered-load split: chunk groups per input direction (pacing)
LOAD_GROUPS = [(0, 1), (1, 4), (4, 8)]


def tf32_round(x):
    if MM_DT == F32:
        return np.asarray(x, np.float32)
    xi = np.asarray(x, np.float32).view(np.uint32)
    xi = (xi + np.uint32(1 << 12)) & np.uint32(0xFFFFE000)
    return xi.view(np.float32)


def build(T=T_STEPS):
    nc = bacc.Bacc("TRN2", num_devices=NCORES)
    RG = [list(range(NCORES))]

    din = dict(kind="ExternalInput")
    w0T = nc.dram_tensor("w0T", [2, 128, 8, 512], MMD, **din)
    w0aug = nc.dram_tensor("w0aug", [2, 2, 512], MMD, **din)
    w1iT = nc.dram_tensor("w1iT", [2, 128, 16, 512], MMD, **din)
    w1hT = nc.dram_tensor("w1hT", [2, 128, 8, 512], MMD, **din)
    b1row = nc.dram_tensor("b1row", [2, 1, 512], MMD, **din)
    linTc = nc.dram_tensor("linTc", [128, 2, 64], MMD, **din)
    linrow = nc.dram_tensor("linrow", [1, 64], MMD, **din)
    iotam = nc.dram_tensor("iotam", [128, 64], F32, **din)
    notkeep = nc.dram_tensor("notkeep", [128, 64], F32, **din)
    hT0 = nc.dram_tensor("hT0", [4, 128, 8, 128], MMD, **din)
    c0s = nc.dram_tensor("c0s", [4, 128, 128], F32, **din)
    onesrow = nc.dram_tensor("onesrow", [1, 128], MMD, **din)
    x0row = nc.dram_tensor("x0row", [1, 128], MMD, **din)
    flag0 = nc.dram_tensor("flag0", [128, 1], F32, **din)
    hT_f = nc.dram_tensor("hT_f", [4, 128, 8, 128], MMD, kind="ExternalOutput")
    c_f = nc.dram_tensor("c_f", [4, 128, 128], F32, kind="ExternalOutput")
    flag_f = nc.dram_tensor("flag_f", [128, 1], F32, kind="ExternalOutput")
    idx_f = nc.dram_tensor("idx_f", [128, 1], F32, kind="ExternalOutput")
    y = nc.dram_tensor("y", [B, T, V], F32, kind="ExternalOutput")

    with tile.TileContext(nc) as tc:
        import contextlib

        ctx = contextlib.ExitStack()
        with ctx:
            wp = ctx.enter_context(tc.tile_pool(name="weights", bufs=1))
            hp = ctx.enter_context(tc.tile_pool(name="hstate", bufs=2))
            cp = ctx.enter_context(tc.tile_pool(name="cstate", bufs=2))
            gp = ctx.enter_context(tc.tile_pool(name="gact", bufs=1))
            ewp = ctx.enter_context(tc.tile_pool(name="ew", bufs=1))
            sp = ctx.enter_context(tc.tile_pool(name="send", bufs=2))
            ap_ = ctx.enter_context(tc.tile_pool(name="amax", bufs=2))
            yp = ctx.enter_context(tc.tile_pool(name="ybuf", bufs=1))
            pg = ctx.enter_context(tc.tile_pool(name="pgates", bufs=1, space="PSUM"))
            pt = ctx.enter_context(tc.tile_pool(name="ptrans", bufs=2, space="PSUM"))
            px = ctx.enter_context(tc.tile_pool(name="pmisc", bufs=1, space="PSUM"))
            dp = ctx.enter_context(tc.tile_pool(name="dram", bufs=2, space="DRAM"))

            # ---- load L0 weights + constants needed by the prologue.
            # The large L1/linear weight loads are deferred until after the
            # AGH0(0) issue so they stream under its collective window
            # instead of serializing ahead of the first step.
            w0_sb, w0a_sb = [], []
            for d in range(2):
                t_ = wp.tile([128, 8, 512], MMD, tag=f"w0_{d}")
                nc.sync.dma_start(out=t_[:], in_=w0T[d])
                w0_sb.append(t_)
                tb = wp.tile([1, 512], MMD, tag=f"w0b_{d}")
                nc.sync.dma_start(out=tb[:], in_=w0aug[d, 1:2])
                tx = wp.tile([1, 512], MMD, tag=f"w0x_{d}")
                nc.sync.dma_start(out=tx[:], in_=w0aug[d, 0:1])
                w0a_sb.append((tx, tb))
            ident = wp.tile([128, 128], F32, tag="ident")
            make_identity(nc, ident[:])
            ones = wp.tile([1, 128], MMD, tag="ones")
            nc.sync.dma_start(out=ones[:], in_=onesrow[:])

            # ---- initial state ----
            h_prev = []
            for cell in range(4):
                t_ = hp.tile([128, 8, 128], MMD, tag=f"h{cell}")
                nc.sync.dma_start(out=t_[:], in_=hT0[cell])
                h_prev.append(t_)
            c_prev = []
            for cell in range(4):
                t_ = cp.tile([128, 128], F32, tag=f"c{cell}")
                nc.sync.dma_start(out=t_[:], in_=c0s[cell])
                c_prev.append(t_)
            flag_prev = ap_.tile([128, 1], F32, tag="flag")
            nc.sync.dma_start(out=flag_prev[:], in_=flag0[:])
            x_row = ap_.tile([1, 128], MMD, tag="xrow")
            nc.sync.dma_start(out=x_row[:], in_=x0row[:])

            def lstm_ew_pre(g, c_in, cell):
                """gate PSUM [128,512] (i,f,o,g blocks) + c_in -> (cn, h2),
                Act/DVE only (no PE ops)."""
                a = gp.tile([128, 512], F32, tag=f"a{cell}")
                nc.scalar.activation(a[:, 0:384], g[:, 0:384], AF.Sigmoid)
                nc.scalar.activation(a[:, 384:512], g[:, 384:512], AF.Tanh)
                t1 = ewp.tile([128, 128], F32, tag=f"t1_{cell}")
                nc.vector.tensor_mul(t1[:], a[:, 128:256], c_in[:])
                t2 = ewp.tile([128, 128], F32, tag=f"t2_{cell}")
                nc.vector.tensor_mul(t2[:], a[:, 0:128], a[:, 384:512])
                cn = cp.tile([128, 128], F32, tag=f"c{cell}")
                nc.vector.tensor_add(cn[:], t1[:], t2[:])
                tc2 = ewp.tile([128, 128], F32, tag=f"tc2_{cell}")
                nc.scalar.activation(tc2[:], cn[:], AF.Tanh)
                h2 = gp.tile([128, 128], F32, tag=f"h2_{cell}")
                nc.vector.tensor_mul(h2[:], a[:, 256:384], tc2[:])
                return cn, h2

            def h_transpose(h2, dst, dst_col):
                ht = pt.tile([128, 128], F32, tag="ht")
                nc.tensor.transpose(ht[:], h2[:], ident[:])
                nc.vector.tensor_copy(dst[:, dst_col:dst_col + 128], ht[:])

            def lstm_ew(g, c_in, cell, dst, dst_col):
                cn, h2 = lstm_ew_pre(g, c_in, cell)
                h_transpose(h2, dst, dst_col)
                return cn

            # ---- prologue: L0(0) + stage + AGH0(0) ----
            g0 = []
            for d in range(2):
                g = pg.tile([128, 512], F32, tag=f"g0{d}")
                nc.tensor.matmul(g[:], (ones[:]), (w0a_sb[d][1][:]),
                                 start=True, stop=False)
                for k in range(8):
                    nc.tensor.matmul(g[:], (h_prev[d][:, k, :]),
                                     (w0_sb[d][:, k, :]),
                                     start=False, stop=False)
                nc.tensor.matmul(g[:], (x_row[:]), (w0a_sb[d][0][:]),
                                 start=False, stop=True)
                g0.append(g)
            sendA = sp.tile([128, 256], MMD, tag="sendA")
            agA_in = dp.tile([128, 256], MMD, tag="agAi")
            c_new = [None] * 4
            for d in range(2):
                c_new[d] = lstm_ew(g0[d], c_prev[d], d, sendA, d * 128)
                nc.sync.dma_start(out=agA_in[:, d * 128:(d + 1) * 128],
                                  in_=sendA[:, d * 128:(d + 1) * 128])
            c_prev = [c_new[0], c_new[1], c_prev[2], c_prev[3]]
            agA_out = dp.tile([1024, 256], MMD, tag="agAo", addr_space="Shared")
            nc.gpsimd.collective_compute(
                "AllGather", ALU.bypass, replica_groups=RG,
                ins=[agA_in.opt()], outs=[agA_out.opt()],
            )

            # ---- deferred L1/linear weight loads (hide under AGH0(0)) ----
            w1i_sb, w1h_sb, b1_sb = [], [], []
            for d in range(2):
                t_ = wp.tile([1, 512], MMD, tag=f"b1_{d}")
                nc.sync.dma_start(out=t_[:], in_=b1row[d])
                b1_sb.append(t_)
            for d in range(2):
                t_ = wp.tile([128, 8, 512], MMD, tag=f"w1h_{d}")
                nc.sync.dma_start(out=t_[:], in_=w1hT[d])
                w1h_sb.append(t_)
            for d in range(2):
                t_ = wp.tile([128, 16, 512], MMD, tag=f"w1i_{d}")
                nc.sync.dma_start(out=t_[:], in_=w1iT[d])
                w1i_sb.append(t_)
            lin_sb = wp.tile([128, 2, 64], MMD, tag="lin")
            nc.sync.dma_start(out=lin_sb[:], in_=linTc[:])
            linr_sb = wp.tile([1, 64], MMD, tag="linr")
            nc.sync.dma_start(out=linr_sb[:], in_=linrow[:])
            iot_sb = wp.tile([128, 64], F32, tag="iot")
            nc.sync.dma_start(out=iot_sb[:], in_=iotam[:])
            nk_sb = wp.tile([128, 64], F32, tag="nk")
            nc.sync.dma_start(out=nk_sb[:], in_=notkeep[:])

            idx = None
            ybuf = None
            agB_outs = [None, None]

            for t in range(T):
                last = t == T - 1
                # -- 1) h1T-d0(t-1) gathered load (skip t=0: prologue loaded)
                if t > 0:
                    t_ = hp.tile([128, 8, 128], MMD, tag="h2")
                    nc.sync.dma_start(
                        out=t_[:],
                        in_=agB_outs[0][:, 0:128].rearrange(
                            "(k p) b -> p k b", p=128),
                    )
                    h_prev[2] = t_
                # -- 2) g1(t) partials: bias both dirs + w_hh1 dir0
                #       (hh-d0 hides in the AGH0(t) window)
                g1 = []
                for d in range(2):
                    g = pg.tile([128, 512], F32, tag=f"g1{d}")
                    nc.tensor.matmul(g[:], (ones[:]), (b1_sb[d][:]),
                                     start=True, stop=False)
                    g1.append(g)
                for k in range(8):
                    nc.tensor.matmul(g1[0][:], (h_prev[2][:, k, :]),
                                     (w1h_sb[0][:, k, :]),
                                     start=False, stop=False)
                # -- 3) h0T(t) gathered loads, split for progressive pacing
                h0n = []
                for d in range(2):
                    t_ = hp.tile([128, 8, 128], MMD, tag=f"h{d}")
                    for k0, k1 in LOAD_GROUPS:
                        nc.sync.dma_start(
                            out=t_[:, k0:k1, :],
                            in_=agA_out[k0 * 128:k1 * 128,
                                        d * 128:(d + 1) * 128].rearrange(
                                "(k p) b -> p k b", p=128),
                        )
                    h0n.append(t_)
                    h_prev[d] = t_
                # -- 4) ih(t) for both output dirs (hides in AGH1b(t-1) win)
                for d in range(2):
                    for sd in range(2):
                        for k0, k1 in LOAD_GROUPS:
                            for k in range(k0, k1):
                                lastmm = d == 0 and sd == 1 and k == 7
                                nc.tensor.matmul(
                                    g1[d][:], (h0n[sd][:, k, :]),
                                    (w1i_sb[d][:, sd * 8 + k, :]),
                                    start=False, stop=lastmm,
                                )
                # -- 5a) ew-d0 + transpose + logit partial half 0 + AR0: all
                #        hide in the AGH1b(t-1) window (ready after ih stop)
                sendB = sp.tile([128, 256], MMD, tag="sendB")
                lpt = px.tile([128, 128], F32, tag="lp")
                lp0 = lpt[:, 0:64]
                lp1 = lpt[:, 64:128]
                nc.tensor.matmul(lp0[:], (ones[:]), (linr_sb[:]),
                                 start=True, stop=False)
                c_new[2], h2_0 = lstm_ew_pre(g1[0], c_prev[2], 2)
                h_transpose(h2_0, sendB, 0)
                nc.tensor.matmul(lp0[:], (sendB[:, 0:128]), (lin_sb[:, 0, :]),
                                 start=False, stop=True)
                agB0_in = dp.tile([128, 129], MMD, tag="agB0i")
                nc.sync.dma_start(out=agB0_in[:, 0:128], in_=sendB[:, 0:128])
                sendBl0 = sp.tile([128, 64], F32, tag="sendBl0")
                nc.vector.tensor_copy(sendBl0[:], lp0[:])
                ar0_in = dp.tile([128, 64], F32, tag="ar0i")
                nc.sync.dma_start(out=ar0_in[:], in_=sendBl0[:])
                ar0_out = dp.tile([128, 64], F32, tag="ar0o",
                                  addr_space="Shared")
                nc.gpsimd.collective_compute(
                    "AllReduce", ALU.add, replica_groups=RG,
                    ins=[ar0_in.opt()], outs=[ar0_out.opt()],
                )
                # -- 4b) h1T-d1(t-1) load (paced) + hh-d1(t) + ew-d1 + AR1:
                #        this tail hides in the AR0(t) window
                if t > 0:
                    t_ = hp.tile([128, 8, 128], MMD, tag="h3")
                    for k0, k1 in LOAD_GROUPS:
                        nc.sync.dma_start(
                            out=t_[:, k0:k1, :],
                            in_=agB_outs[1][k0 * 128:k1 * 128, 0:128].rearrange(
                                "(k p) b -> p k b", p=128),
                        )
                    h_prev[3] = t_
                for k0, k1 in LOAD_GROUPS:
                    for k in range(k0, k1):
                        nc.tensor.matmul(g1[1][:], (h_prev[3][:, k, :]),
                                         (w1h_sb[1][:, k, :]),
                                         start=False, stop=(k == 7))
                c_new[3], h2_1 = lstm_ew_pre(g1[1], c_prev[3], 3)
                h_transpose(h2_1, sendB, 128)
                nc.tensor.matmul(lp1[:], (sendB[:, 128:256]), (lin_sb[:, 1, :]),
                                 start=True, stop=True)
                c_prev = [c_prev[0], c_prev[1], c_new[2], c_new[3]]
                sendBl1 = sp.tile([128, 64], F32, tag="sendBl1")
                nc.vector.tensor_copy(sendBl1[:], lp1[:])
                ar1_in = dp.tile([128, 64], F32, tag="ar1i")
                nc.sync.dma_start(out=ar1_in[:], in_=sendBl1[:])
                agB1_in = dp.tile([128, 129], MMD, tag="agB1i")
                nc.sync.dma_start(out=agB1_in[:, 0:128], in_=sendB[:, 128:256])
                # -- 7) AR1(t) + AGH1a(t). Device order forced by dep DMAs:
                # AR1 before AGH1a (d2d col from ar1_in), AGH1b after
                # AGH0(t+1) (d2d col from agA_in).
                ar1_out = dp.tile([128, 64], F32, tag="ar1o",
                                  addr_space="Shared")
                nc.gpsimd.collective_compute(
                    "AllReduce", ALU.add, replica_groups=RG,
                    ins=[ar1_in.opt()], outs=[ar1_out.opt()],
                )
                nc.sync.dma_start(out=agB0_in[:, 128:129].bitcast(F32),
                                  in_=ar1_in[:, 0:1])
                agB0_out = dp.tile([1024, 129], MMD, tag="agB0o",
                                   addr_space="Shared")
                nc.gpsimd.collective_compute(
                    "AllGather", ALU.bypass, replica_groups=RG,
                    ins=[agB0_in.opt()], outs=[agB0_out.opt()],
                )
                agB_outs = [agB0_out, None]
                # -- 8) g0(t+1) partials: bias + w_hh0 (hidden in AR window)
                if not last:
                    g0 = []
                    for d in range(2):
                        g = pg.tile([128, 512], F32, tag=f"g0{d}")
                        nc.tensor.matmul(g[:], (ones[:]), (w0a_sb[d][1][:]),
                                         start=True, stop=False)
                        for k in range(8):
                            nc.tensor.matmul(g[:], (h0n[d][:, k, :]),
                                             (w0_sb[d][:, k, :]),
                                             start=False, stop=False)
                        g0.append(g)
                # -- 9) logits load (both AR halves) + argmax chain
                L0h = ap_.tile([128, 64], F32, tag="L0h")
                nc.sync.dma_start(out=L0h[:], in_=ar0_out[:])
                L1h = ap_.tile([128, 64], F32, tag="L1h")
                nc.sync.dma_start(out=L1h[:], in_=ar1_out[:])
                L = ap_.tile([128, 64], F32, tag="L")
                nc.vector.tensor_add(L[:], L0h[:], L1h[:])
                m = ap_.tile([128, 1], F32, tag="m")
                nc.vector.tensor_reduce(m[:], L[:], axis=mybir.AxisListType.X,
                                        op=ALU.max)
                ismax = ap_.tile([128, 64], F32, tag="ismax")
                nc.vector.tensor_scalar(ismax[:], L[:], m[:], None,
                                        op0=ALU.is_ge)
                cand = ap_.tile([128, 64], F32, tag="cand")
                nc.vector.tensor_mul(cand[:], ismax[:], iot_sb[:])
                idxm = ap_.tile([128, 1], F32, tag="idxm")
                nc.vector.tensor_reduce(idxm[:], cand[:],
                                        axis=mybir.AxisListType.X, op=ALU.min)
                idx = ap_.tile([128, 1], F32, tag="idx")
                nc.vector.tensor_scalar(idx[:], idxm[:], 100.0, None,
                                        op0=ALU.add)
                # -- 10) x feedback + close g0(t+1) (hidden in AGH1 window)
                if not last:
                    x_ps = px.tile([1, 128], F32, tag="xps")
                    nc.tensor.transpose(x_ps[:], idx[:], ident[:])
                    x_row = ap_.tile([1, 128], MMD, tag="xrow")
                    nc.vector.tensor_copy(x_row[:], x_ps[:])
                    for d in range(2):
                        nc.tensor.matmul(g0[d][:], (x_row[:]),
                                         (w0a_sb[d][0][:]),
                                         start=False, stop=True)
                # -- 11) flag + masked store
                flagb = ap_.tile([128, 1], F32, tag="flagb")
                nc.vector.tensor_scalar(flagb[:], idx[:], 1.0, None,
                                        op0=ALU.is_equal)
                fnew = ap_.tile([128, 1], F32, tag="flag")
                nc.vector.tensor_max(fnew[:], flag_prev[:], flagb[:])
                tk = ap_.tile([128, 64], F32, tag="tk")
                nc.vector.tensor_mul(tk[:], L[:], nk_sb[:])
                tk2 = ap_.tile([128, 64], F32, tag="tk2")
                nc.vector.tensor_scalar(tk2[:], tk[:], fnew[:], None,
                                        op0=ALU.mult)
                if t % CHUNK == 0:
                    ybuf = yp.tile([128, CHUNK, 64], F32, tag="ybuf")
                nc.vector.tensor_sub(ybuf[:, t % CHUNK, :], L[:], tk2[:])
                if t % CHUNK == CHUNK - 1:
                    nc.sync.dma_start(out=y[:, t - CHUNK + 1:t + 1, :],
                                      in_=ybuf[:])
                flag_prev = fnew
                # -- 12) L0(t+1) elementwise + stage + AGH0(t+1) + AGH1b(t)
                if not last:
                    sendA = sp.tile([128, 256], MMD, tag="sendA")
                    agA_in = dp.tile([128, 256], MMD, tag="agAi")
                    for d in range(2):
                        c_new[d] = lstm_ew(g0[d], c_prev[d], d, sendA, d * 128)
                    nc.sync.dma_start(out=agA_in[:], in_=sendA[:])
                    c_prev = [c_new[0], c_new[1], c_prev[2], c_prev[3]]
                    agA_out = dp.tile([1024, 256], MMD, tag="agAo",
                                      addr_space="Shared")
                    nc.gpsimd.collective_compute(
                        "AllGather", ALU.bypass, replica_groups=RG,
                        ins=[agA_in.opt()], outs=[agA_out.opt()],
                    )
                    # dep col: AGH1b(t) becomes ready only after agA(t+1)
                    nc.sync.dma_start(out=agB1_in[:, 128:129],
                                      in_=agA_in[:, 0:1])
                else:
                    nc.sync.dma_start(out=agB1_in[:, 128:129].bitcast(F32),
                                      in_=ar1_in[:, 0:1])
                agB1_out = dp.tile([1024, 129], MMD, tag="agB1o",
                                   addr_space="Shared")
                nc.gpsimd.collective_compute(
                    "AllGather", ALU.bypass, replica_groups=RG,
                    ins=[agB1_in.opt()], outs=[agB1_out.opt()],
                )
                agB_outs = [agB0_out, agB1_out]
            # ---- epilogue ----
            if T % CHUNK != 0:
                nfin = T % CHUNK
                nc.sync.dma_start(out=y[:, T - nfin:T, :], in_=ybuf[:, 0:nfin, :])
            for d in range(2):
                t_ = hp.tile([128, 8, 128], MMD, tag=f"h{2 + d}")
                nc.sync.dma_start(
                    out=t_[:],
                    in_=agB_outs[d][:, 0:128].rearrange(
                        "(k p) b -> p k b", p=128),
                )
                h_prev[2 + d] = t_
            for cell in range(4):
                nc.sync.dma_start(out=hT_f[cell], in_=h_prev[cell][:])
                nc.sync.dma_start(out=c_f[cell], in_=c_prev[cell][:])
            nc.sync.dma_start(out=flag_f[:], in_=flag_prev[:])
            nc.sync.dma_start(out=idx_f[:], in_=idx[:])
    nc.finalize()
    return nc


def prep_inputs(h0, c0, w_ih0, w_hh0, b0, w_ih1, w_hh1, b1, lin_w, lin_b):
    """Host-side packing: per-core sliced/transposed weight + state arrays."""
    h0 = np.asarray(h0, np.float32).reshape(2, 2, B, H)
    c0 = np.asarray(c0, np.float32).reshape(2, 2, B, H)
    w_ih0 = np.asarray(w_ih0, np.float32)
    w_hh0 = np.asarray(w_hh0, np.float32)
    b0 = np.asarray(b0, np.float32)
    w_ih1 = np.asarray(w_ih1, np.float32)
    w_hh1 = np.asarray(w_hh1, np.float32)
    b1 = np.asarray(b1, np.float32)
    lin_w = np.asarray(lin_w, np.float32)
    lin_b = np.asarray(lin_b, np.float32)

    iota = np.broadcast_to((np.arange(V) - 100.0).astype(np.float32),
                           (128, V)).copy()
    nk = np.ones((128, V), np.float32)
    nk[:, KEEP_IDX] = 0.0

    hT0 = np.zeros((4, 128, 8, B), np.float32)
    for l in range(2):
        for d in range(2):
            cell = l * 2 + d
            hT0[cell] = h0[l, d].T.reshape(8, 128, B).transpose(1, 0, 2)

    in_maps = []
    for c in range(NCORES):
        rows = np.concatenate([np.arange(gb + c * 128, gb + c * 128 + 128)
                               for gb in GBASE])

        def packT(w, kt):
            # w: (4H, K*128) -> select rows -> [p, k, n]
            sel = w[rows, :]  # (512, kt*128)
            return np.ascontiguousarray(
                sel.reshape(512, kt, 128).transpose(2, 1, 0))

        w0T = np.stack([packT(w_hh0[d], 8) for d in range(2)])
        w1iT = np.stack([packT(w_ih1[d], 16) for d in range(2)])
        w1hT = np.stack([packT(w_hh1[d], 8) for d in range(2)])
        w0aug = np.stack([np.stack([w_ih0[d][rows, 0], b0[d][rows]])
                          for d in range(2)])
        b1row = np.stack([b1[d][rows][None, :] for d in range(2)])
        linTc = np.stack(
            [lin_w[:, c * 128:(c + 1) * 128].T,
             lin_w[:, H + c * 128:H + (c + 1) * 128].T], axis=1)
        c0slice = np.zeros((4, 128, 128), np.float32)
        for l in range(2):
            for d in range(2):
                c0slice[l * 2 + d] = c0[l, d][:, c * 128:(c + 1) * 128]
        in_maps.append({
            "w0T": tf32_round(np.ascontiguousarray(w0T)),
            "w0aug": tf32_round(np.ascontiguousarray(w0aug)),
            "w1iT": tf32_round(np.ascontiguousarray(w1iT)),
            "w1hT": tf32_round(np.ascontiguousarray(w1hT)),
            "b1row": tf32_round(np.ascontiguousarray(b1row)),
            "linTc": tf32_round(np.ascontiguousarray(linTc)),
            "linrow": tf32_round((lin_b / NCORES)[None, :]),
            "iotam": iota,
            "notkeep": nk,
            "hT0": tf32_round(hT0),
            "c0s": np.ascontiguousarray(c0slice),
            "onesrow": np.ones((1, 128), np.float32),
            "x0row": np.full((1, 128), MASK_IDX, np.float32),
            "flag0": np.zeros((128, 1), np.float32),
        })
    return in_maps


_NC_CACHE = {}


def _get_nc(T):
    if T not in _NC_CACHE:
        _NC_CACHE[T] = build(T)
    return _NC_CACHE[T]


T_LAUNCH = 256


def kernel(h0, c0, w_ih0, w_hh0, b0, w_ih1, w_hh1, b1, lin_w, lin_b,
           decoder_output_length, batch_size, _want_results=False):
    T = int(decoder_output_length)
    assert int(batch_size) == B
    in_maps = prep_inputs(h0, c0, w_ih0, w_hh0, b0, w_ih1, w_hh1, b1,
                          lin_w, lin_b)
    chunks = []
    t_done = 0
    res = None
    while t_done < T:
        t_this = min(T_LAUNCH, T - t_done)
        nc = _get_nc(t_this)
        res = bass_utils.run_bass_kernel_spmd(nc, in_maps,
                                              core_ids=list(range(NCORES)))
        chunks.append(res.results[0]["y"])
        t_done += t_this
        if t_done < T:
            idxs = res.results[0]["idx_f"]  # (128,1) float indices
            xrow = np.ascontiguousarray(idxs.reshape(1, 128))
            for c in range(NCORES):
                rc = res.results[c]
                in_maps[c] = dict(in_maps[c])
                in_maps[c]["hT0"] = rc["hT_f"]
                in_maps[c]["c0s"] = rc["c_f"]
                in_maps[c]["flag0"] = rc["flag_f"]
                in_maps[c]["x0row"] = xrow
    out = np.concatenate(chunks, axis=1) if len(chunks) > 1 else chunks[0]
    if _want_results:
        return out, res
    return out


# revision 25
# speedup vs baseline: 2.2216x; 1.0042x over previous
"""Trainium2 Bass kernel for nn_Decoder (2-layer bidirectional LSTM decoder,
autoregressive argmax feedback, T=512 steps, B=128, H=1024, V=64).

Strategy: 8-way tensor parallel over the 4H gate dimension. Each core holds a
512-wide slice of every gate projection (re-ordered [i,f,o,g] so activations
fuse), keeps the LSTM recurrence state resident, and exchanges state via five
collectives per step, scheduled back-to-back so that every piece of compute
hides inside some collective's window (collectives run on dedicated cores
concurrently with the engines):
  AR0    AllReduce of logit partial half 0 (lin_b/8 + dir-0 h1 term)
         -> its window covers the hh-d1 matmuls + dir-1 elementwise tail
  AR1    AllReduce of logit partial half 1 (dir-1 h1 term)
         -> its window covers next step's L0 h-recurrence partials
  AGH1a  AllGather of h1T dir-0 slices
         -> its window covers argmax -> x -> L0 close/elementwise/staging
  AGH0   AllGather of h0T(t+1) slices (both dirs)
         -> its window covers the hh-d0 matmuls
  AGH1b  AllGather of h1T dir-1 slices
         -> its window covers the 32 L1 input-projection matmuls + ew-d0
Logits are AR0out + AR1out. Device order is forced where the readiness-based
tile scheduler would invert it, via tiny DRAM->DRAM dependency-column DMAs
(AGH1a only becomes ready after AR1's input is staged; AGH1b only after
AGH0(t+1)'s input is staged). The bidirectional split works because each
LSTM direction's h-recurrence contracts only its own direction's h state.
Gathered-state DMA loads are split (igniter chunk first) so dependent matmul
groups become runnable progressively, keeping the PE p-state ramped.
"""

import os
import sys

import numpy as np

sys.path.insert(0, "/opt/trn_rl_repo")

import concourse.bass as bass  # noqa: E402
import concourse.mybir as mybir  # noqa: E402
import concourse.tile as tile  # noqa: E402
from concourse import bacc  # noqa: E402
from concourse import bass_utils  # noqa: E402
from concourse.masks import make_identity  # noqa: E402

H = 1024
V = 64
B = 128
NCORES = 8
MASK_IDX = 4.0
KEEP_IDX = 3
T_STEPS = int(os.environ.get("DEC_T", "512"))
CHUNK = int(os.environ.get("DEC_CHUNK", "8"))
MM_DT = mybir.dt.float32r if os.environ.get("DEC_MMDT", "fp32r") == "fp32r" else mybir.dt.float32
F32 = mybir.dt.float32
MMD = MM_DT
AF = mybir.ActivationFunctionType
ALU = mybir.AluOpType

# gate blocks packed per-core as [i, f, o, g] (torch order in rows is i,f,g,o)
GBASE = [0, H, 3 * H, 2 * H]

# h0T gathered-load split: chunk groups per input direction (pacing)
LOAD_GROUPS = [(0, 1), (1, 4), (4, 8)]


def tf32_round(x):
    if MM_DT == F32:
        return np.asarray(x, np.float32)
    xi = np.asarray(x, np.float32).view(np.uint32)
    xi = (xi + np.uint32(1 << 12)) & np.uint32(0xFFFFE000)
    return xi.view(np.float32)


def build(T=T_STEPS):
    nc = bacc.Bacc("TRN2", num_devices=NCORES)
    RG = [list(range(NCORES))]

    din = dict(kind="ExternalInput")
    w0T = nc.dram_tensor("w0T", [2, 128, 8, 512], MMD, **din)
    w0aug = nc.dram_tensor("w0aug", [2, 2, 512], MMD, **din)
    w1iT = nc.dram_tensor("w1iT", [2, 128, 16, 512], MMD, **din)
    w1hT = nc.dram_tensor("w1hT", [2, 128, 8, 512], MMD, **din)
    b1row = nc.dram_tensor("b1row", [2, 1, 512], MMD, **din)
    linTc = nc.dram_tensor("linTc", [128, 2, 64], MMD, **din)
    linrow = nc.dram_tensor("linrow", [1, 64], MMD, **din)
    iotam = nc.dram_tensor("iotam", [128, 64], F32, **din)
    notkeep = nc.dram_tensor("notkeep", [128, 64], F32, **din)
    hT0 = nc.dram_tensor("hT0", [4, 128, 8, 128], MMD, **din)
    c0s = nc.dram_tensor("c0s", [4, 128, 128], F32, **din)
    onesrow = nc.dram_tensor("onesrow", [1, 128], MMD, **din)
    x0row = nc.dram_tensor("x0row", [1, 128], MMD, **din)
    flag0 = nc.dram_tensor("flag0", [128, 1], F32, **din)
    hT_f = nc.dram_tensor("hT_f", [4, 128, 8, 128], MMD, kind="ExternalOutput")
    c_f = nc.dram_tensor("c_f", [4, 128, 128], F32, kind="ExternalOutput")
    flag_f = nc.dram_tensor("flag_f", [128, 1], F32, kind="ExternalOutput")
    idx_f = nc.dram_tensor("idx_f", [128, 1], F32, kind="ExternalOutput")
    y = nc.dram_tensor("y", [B, T, V], F32, kind="ExternalOutput")

    with tile.TileContext(nc) as tc:
        import contextlib

        ctx = contextlib.ExitStack()
        with ctx:
            wp = ctx.enter_context(tc.tile_pool(name="weights", bufs=1))
            hp = ctx.enter_context(tc.tile_pool(name="hstate", bufs=2))
            cp = ctx.enter_context(tc.tile_pool(name="cstate", bufs=2))
            gp = ctx.enter_context(tc.tile_pool(name="gact", bufs=1))
            ewp = ctx.enter_context(tc.tile_pool(name="ew", bufs=1))
            sp = ctx.enter_context(tc.tile_pool(name="send", bufs=2))
            ap_ = ctx.enter_context(tc.tile_pool(name="amax", bufs=2))
            yp = ctx.enter_context(tc.tile_pool(name="ybuf", bufs=1))
            pg = ctx.enter_context(tc.tile_pool(name="pgates", bufs=1, space="PSUM"))
            pt = ctx.enter_context(tc.tile_pool(name="ptrans", bufs=2, space="PSUM"))
            px = ctx.enter_context(tc.tile_pool(name="pmisc", bufs=1, space="PSUM"))
            dp = ctx.enter_context(tc.tile_pool(name="dram", bufs=2, space="DRAM"))

            # ---- load L0 weights + constants needed by the prologue.
            # The large L1/linear weight loads are deferred until after the
            # AGH0(0) issue so they stream under its collective window
            # instead of serializing ahead of the first step.
            w0_sb, w0a_sb = [], []
            for d in range(2):
                t_ = wp.tile([128, 8, 512], MMD, tag=f"w0_{d}")
                nc.sync.dma_start(out=t_[:], in_=w0T[d])
                w0_sb.append(t_)
                tb = wp.tile([1, 512], MMD, tag=f"w0b_{d}")
                nc.sync.dma_start(out=tb[:], in_=w0aug[d, 1:2])
                tx = wp.tile([1, 512], MMD, tag=f"w0x_{d}")
                nc.sync.dma_start(out=tx[:], in_=w0aug[d, 0:1])
                w0a_sb.append((tx, tb))
            ident = wp.tile([128, 128], F32, tag="ident")
            make_identity(nc, ident[:])
            ones = wp.tile([1, 128], MMD, tag="ones")
            nc.sync.dma_start(out=ones[:], in_=onesrow[:])

            # ---- initial state ----
            h_prev = []
            for cell in range(4):
                t_ = hp.tile([128, 8, 128], MMD, tag=f"h{cell}")
                nc.sync.dma_start(out=t_[:], in_=hT0[cell])
                h_prev.append(t_)
            c_prev = []
            for cell in range(4):
                t_ = cp.tile([128, 128], F32, tag=f"c{cell}")
                nc.sync.dma_start(out=t_[:], in_=c0s[cell])
                c_prev.append(t_)
            flag_prev = ap_.tile([128, 1], F32, tag="flag")
            nc.sync.dma_start(out=flag_prev[:], in_=flag0[:])
            x_row = ap_.tile([1, 128], MMD, tag="xrow")
            nc.sync.dma_start(out=x_row[:], in_=x0row[:])

            def lstm_ew_pre(g, c_in, cell):
                """gate PSUM [128,512] (i,f,o,g blocks) + c_in -> (cn, h2),
                Act/DVE only (no PE ops)."""
                a = gp.tile([128, 512], F32, tag=f"a{cell}")
                nc.scalar.activation(a[:, 0:384], g[:, 0:384], AF.Sigmoid)
                nc.scalar.activation(a[:, 384:512], g[:, 384:512], AF.Tanh)
                t1 = ewp.tile([128, 128], F32, tag=f"t1_{cell}")
                nc.vector.tensor_mul(t1[:], a[:, 128:256], c_in[:])
                t2 = ewp.tile([128, 128], F32, tag=f"t2_{cell}")
                nc.vector.tensor_mul(t2[:], a[:, 0:128], a[:, 384:512])
                cn = cp.tile([128, 128], F32, tag=f"c{cell}")
                nc.vector.tensor_add(cn[:], t1[:], t2[:])
                tc2 = ewp.tile([128, 128], F32, tag=f"tc2_{cell}")
                nc.scalar.activation(tc2[:], cn[:], AF.Tanh)
                h2 = gp.tile([128, 128], F32, tag=f"h2_{cell}")
                nc.vector.tensor_mul(h2[:], a[:, 256:384], tc2[:])
                return cn, h2

            def h_transpose(h2, dst, dst_col):
                ht = pt.tile([128, 128], F32, tag="ht")
                nc.tensor.transpose(ht[:], h2[:], ident[:])
                nc.vector.tensor_copy(dst[:, dst_col:dst_col + 128], ht[:])

            def lstm_ew(g, c_in, cell, dst, dst_col):
                cn, h2 = lstm_ew_pre(g, c_in, cell)
                h_transpose(h2, dst, dst_col)
                return cn

            # ---- prologue: L0(0) + stage + AGH0(0) ----
            g0 = []
            for d in range(2):
                g = pg.tile([128, 512], F32, tag=f"g0{d}")
                nc.tensor.matmul(g[:], (ones[:]), (w0a_sb[d][1][:]),
                                 start=True, stop=False)
                for k in range(8):
                    nc.tensor.matmul(g[:], (h_prev[d][:, k, :]),
                                     (w0_sb[d][:, k, :]),
                                     start=False, stop=False)
                nc.tensor.matmul(g[:], (x_row[:]), (w0a_sb[d][0][:]),
                                 start=False, stop=True)
                g0.append(g)
            sendA = sp.tile([128, 256], MMD, tag="sendA")
            agA_in = dp.tile([128, 256], MMD, tag="agAi")
            c_new = [None] * 4
            for d in range(2):
                c_new[d] = lstm_ew(g0[d], c_prev[d], d, sendA, d * 128)
                nc.sync.dma_start(out=agA_in[:, d * 128:(d + 1) * 128],
                                  in_=sendA[:, d * 128:(d + 1) * 128])
            c_prev = [c_new[0], c_new[1], c_prev[2], c_prev[3]]
            agA_out = dp.tile([1024, 256], MMD, tag="agAo", addr_space="Shared")
            nc.gpsimd.collective_compute(
                "AllGather", ALU.bypass, replica_groups=RG,
                ins=[agA_in.opt()], outs=[agA_out.opt()],
            )

            # ---- deferred L1/linear weight loads (hide under AGH0(0)) ----
            w1i_sb, w1h_sb, b1_sb = [], [], []
            for d in range(2):
                t_ = wp.tile([1, 512], MMD, tag=f"b1_{d}")
                nc.sync.dma_start(out=t_[:], in_=b1row[d])
                b1_sb.append(t_)
            for d in range(2):
                t_ = wp.tile([128, 8, 512], MMD, tag=f"w1h_{d}")
                nc.sync.dma_start(out=t_[:], in_=w1hT[d])
                w1h_sb.append(t_)
            for d in range(2):
                t_ = wp.tile([128, 16, 512], MMD, tag=f"w1i_{d}")
                nc.sync.dma_start(out=t_[:], in_=w1iT[d])
                w1i_sb.append(t_)
            lin_sb = wp.tile([128, 2, 64], MMD, tag="lin")
            nc.sync.dma_start(out=lin_sb[:], in_=linTc[:])
            linr_sb = wp.tile([1, 64], MMD, tag="linr")
            nc.sync.dma_start(out=linr_sb[:], in_=linrow[:])
            iot_sb = wp.tile([128, 64], F32, tag="iot")
            nc.sync.dma_start(out=iot_sb[:], in_=iotam[:])
            nk_sb = wp.tile([128, 64], F32, tag="nk")
            nc.sync.dma_start(out=nk_sb[:], in_=notkeep[:])

            idx = None
            ybuf = None
            agB_outs = [None, None]

            for t in range(T):
                last = t == T - 1
                # -- 1) h1T-d0(t-1) gathered load (skip t=0: prologue loaded)
                if t > 0:
                    t_ = hp.tile([128, 8, 128], MMD, tag="h2")
                    nc.sync.dma_start(
                        out=t_[:],
                        in_=agB_outs[0][:, 0:128].rearrange(
                            "(k p) b -> p k b", p=128),
                    )
                    h_prev[2] = t_
                # -- 2) g1(t) partials: bias both dirs + w_hh1 dir0
                #       (hh-d0 hides in the AGH0(t) window)
                g1 = []
                for d in range(2):
                    g = pg.tile([128, 512], F32, tag=f"g1{d}")
                    nc.tensor.matmul(g[:], (ones[:]), (b1_sb[d][:]),
                                     start=True, stop=False)
                    g1.append(g)
                for k in range(8):
                    nc.tensor.matmul(g1[0][:], (h_prev[2][:, k, :]),
                                     (w1h_sb[0][:, k, :]),
                                     start=False, stop=False)
                # -- 3) h0T(t) gathered loads, split for progressive pacing
                h0n = []
                for d in range(2):
                    t_ = hp.tile([128, 8, 128], MMD, tag=f"h{d}")
                    for k0, k1 in LOAD_GROUPS:
                        nc.sync.dma_start(
                            out=t_[:, k0:k1, :],
                            in_=agA_out[k0 * 128:k1 * 128,
                                        d * 128:(d + 1) * 128].rearrange(
                                "(k p) b -> p k b", p=128),
                        )
                    h0n.append(t_)
                    h_prev[d] = t_
                # -- 4) ih(t) for both output dirs (hides in AGH1b(t-1) win)
                for d in range(2):
                    for sd in range(2):
                        for k0, k1 in LOAD_GROUPS:
                            for k in range(k0, k1):
                                lastmm = d == 0 and sd == 1 and k == 7
                                nc.tensor.matmul(
                                    g1[d][:], (h0n[sd][:, k, :]),
                                    (w1i_sb[d][:, sd * 8 + k, :]),
                                    start=False, stop=lastmm,
                                )
                # -- 5a) ew-d0 + transpose + logit partial half 0 + AR0: all
                #        hide in the AGH1b(t-1) window (ready after ih stop)
                sendB = sp.tile([128, 256], MMD, tag="sendB")
                lpt = px.tile([128, 128], F32, tag="lp")
                lp0 = lpt[:, 0:64]
                lp1 = lpt[:, 64:128]
                nc.tensor.matmul(lp0[:], (ones[:]), (linr_sb[:]),
                                 start=True, stop=False)
                c_new[2], h2_0 = lstm_ew_pre(g1[0], c_prev[2], 2)
                h_transpose(h2_0, sendB, 0)
                nc.tensor.matmul(lp0[:], (sendB[:, 0:128]), (lin_sb[:, 0, :]),
                                 start=False, stop=True)
                agB0_in = dp.tile([128, 129], MMD, tag="agB0i")
                nc.sync.dma_start(out=agB0_in[:, 0:128], in_=sendB[:, 0:128])
                sendBl0 = sp.tile([128, 64], F32, tag="sendBl0")
                nc.vector.tensor_copy(sendBl0[:], lp0[:])
                ar0_in = dp.tile([128, 64], F32, tag="ar0i")
                nc.sync.dma_start(out=ar0_in[:], in_=sendBl0[:])
                ar0_out = dp.tile([128, 64], F32, tag="ar0o",
                                  addr_space="Shared")
                nc.gpsimd.collective_compute(
                    "AllReduce", ALU.add, replica_groups=RG,
                    ins=[ar0_in.opt()], outs=[ar0_out.opt()],
                )
                # -- 4b) h1T-d1(t-1) load (paced) + hh-d1(t) + ew-d1 + AR1:
                #        this tail hides in the AR0(t) window
                if t > 0:
                    t_ = hp.tile([128, 8, 128], MMD, tag="h3")
                    for k0, k1 in LOAD_GROUPS:
                        nc.sync.dma_start(
                            out=t_[:, k0:k1, :],
                            in_=agB_outs[1][k0 * 128:k1 * 128, 0:128].rearrange(
                                "(k p) b -> p k b", p=128),
                        )
                    h_prev[3] = t_
                for k0, k1 in LOAD_GROUPS:
                    for k in range(k0, k1):
                        nc.tensor.matmul(g1[1][:], (h_prev[3][:, k, :]),
                                         (w1h_sb[1][:, k, :]),
                                         start=False, stop=(k == 7))
                c_new[3], h2_1 = lstm_ew_pre(g1[1], c_prev[3], 3)
                h_transpose(h2_1, sendB, 128)
                nc.tensor.matmul(lp1[:], (sendB[:, 128:256]), (lin_sb[:, 1, :]),
                                 start=True, stop=True)
                c_prev = [c_prev[0], c_prev[1], c_new[2], c_new[3]]
                sendBl1 = sp.tile([128, 64], F32, tag="sendBl1")
                nc.vector.tensor_copy(sendBl1[:], lp1[:])
                ar1_in = dp.tile([128, 64], F32, tag="ar1i")
                nc.sync.dma_start(out=ar1_in[:], in_=sendBl1[:])
                agB1_in = dp.tile([128, 129], MMD, tag="agB1i")
                nc.sync.dma_start(out=agB1_in[:, 0:128], in_=sendB[:, 128:256])
                # -- 7) AR1(t) + AGH1a(t). Device order forced by dep DMAs:
                # AR1 before AGH1a (d2d col from ar1_in), AGH1b after
                # AGH0(t+1) (d2d col from agA_in).
                ar1_out = dp.tile([128, 64], F32, tag="ar1o",
                                  addr_space="Shared")
                nc.gpsimd.collective_compute(
                    "AllReduce", ALU.add, replica_groups=RG,
                    ins=[ar1_in.opt()], outs=[ar1_out.opt()],
                )
                nc.sync.dma_start(out=agB0_in[:, 128:129].bitcast(F32),
                                  in_=ar1_in[:, 0:1])
                agB0_out = dp.tile([1024, 129], MMD, tag="agB0o",
                                   addr_space="Shared")
                nc.gpsimd.collective_compute(
                    "AllGather", ALU.bypass, replica_groups=RG,
                    ins=[agB0_in.opt()], outs=[agB0_out.opt()],
                )
                agB_outs = [agB0_out, None]
                # -- 8) g0(t+1) partials: bias + w_hh0 (hidden in AR window)
                if not last:
                    g0 = []
                    for d in range(2):
                        g = pg.tile([128, 512], F32, tag=f"g0{d}")
                        nc.tensor.matmul(g[:], (ones[:]), (w0a_sb[d][1][:]),
                                         start=True, stop=False)
                        for k in range(8):
                            nc.tensor.matmul(g[:], (h0n[d][:, k, :]),
                                             (w0_sb[d][:, k, :]),
                                             start=False, stop=False)
                        g0.append(g)
                # -- 9) logits load (both AR halves) + argmax chain
                L0h = ap_.tile([128, 64], F32, tag="L0h")
                nc.sync.dma_start(out=L0h[:], in_=ar0_out[:])
                L1h = ap_.tile([128, 64], F32, tag="L1h")
                nc.sync.dma_start(out=L1h[:], in_=ar1_out[:])
                L = ap_.tile([128, 64], F32, tag="L")
                nc.vector.tensor_add(L[:], L0h[:], L1h[:])
                m = ap_.tile([128, 1], F32, tag="m")
                nc.vector.tensor_reduce(m[:], L[:], axis=mybir.AxisListType.X,
                                        op=ALU.max)
                ismax = ap_.tile([128, 64], F32, tag="ismax")
                nc.vector.tensor_scalar(ismax[:], L[:], m[:], None,
                                        op0=ALU.is_ge)
                cand = ap_.tile([128, 64], F32, tag="cand")
                nc.vector.tensor_mul(cand[:], ismax[:], iot_sb[:])
                idxm = ap_.tile([128, 1], F32, tag="idxm")
                nc.vector.tensor_reduce(idxm[:], cand[:],
                                        axis=mybir.AxisListType.X, op=ALU.min)
                idx = ap_.tile([128, 1], F32, tag="idx")
                nc.vector.tensor_scalar(idx[:], idxm[:], 100.0, None,
                                        op0=ALU.add)
                # -- 10) x feedback + close g0(t+1) (hidden in AGH1 window)
                if not last:
                    x_ps = px.tile([1, 128], F32, tag="xps")
                    nc.tensor.transpose(x_ps[:], idx[:], ident[:])
                    x_row = ap_.tile([1, 128], MMD, tag="xrow")
                    nc.vector.tensor_copy(x_row[:], x_ps[:])
                    for d in range(2):
                        nc.tensor.matmul(g0[d][:], (x_row[:]),
                                         (w0a_sb[d][0][:]),
                                         start=False, stop=True)
                # -- 11) flag + masked store
                flagb = ap_.tile([128, 1], F32, tag="flagb")
                nc.vector.tensor_scalar(flagb[:], idx[:], 1.0, None,
                                        op0=ALU.is_equal)
                fnew = ap_.tile([128, 1], F32, tag="flag")
                nc.vector.tensor_max(fnew[:], flag_prev[:], flagb[:])
                tk = ap_.tile([128, 64], F32, tag="tk")
                nc.vector.tensor_mul(tk[:], L[:], nk_sb[:])
                tk2 = ap_.tile([128, 64], F32, tag="tk2")
                nc.vector.tensor_scalar(tk2[:], tk[:], fnew[:], None,
                                        op0=ALU.mult)
                if t % CHUNK == 0:
                    ybuf = yp.tile([128, CHUNK, 64], F32, tag="ybuf")
                nc.vector.tensor_sub(ybuf[:, t % CHUNK, :], L[:], tk2[:])
                if t % CHUNK == CHUNK - 1:
                    nc.sync.dma_start(out=y[:, t - CHUNK + 1:t + 1, :],
                                      in_=ybuf[:])
                flag_prev = fnew
                # -- 12) L0(t+1) elementwise + stage + AGH0(t+1) + AGH1b(t)
                if not last:
                    sendA = sp.tile([128, 256], MMD, tag="sendA")
                    agA_in = dp.tile([128, 256], MMD, tag="agAi")
                    for d in range(2):
                        c_new[d] = lstm_ew(g0[d], c_prev[d], d, sendA, d * 128)
                    nc.sync.dma_start(out=agA_in[:], in_=sendA[:])
                    c_prev = [c_new[0], c_new[1], c_prev[2], c_prev[3]]
                    agA_out = dp.tile([1024, 256], MMD, tag="agAo",
                                      addr_space="Shared")
                    nc.gpsimd.collective_compute(
                        "AllGather", ALU.bypass, replica_groups=RG,
                        ins=[agA_in.opt()], outs=[agA_out.opt()],
                    )
                    # dep col: AGH1b(t) becomes ready only after agA(t+1)
                    nc.sync.dma_start(out=agB1_in[:, 128:129],
                                      in_=agA_in[:, 0:1])
                else:
                    nc.sync.dma_start(out=agB1_in[:, 128:129].bitcast(F32),
                                      in_=ar1_in[:, 0:1])
                agB1_out = dp.tile([1024, 129], MMD, tag="agB1o",
                                   addr_space="Shared")
                nc.gpsimd.collective_compute(
                    "AllGather", ALU.bypass, replica_groups=RG,
                    ins=[agB1_in.opt()], outs=[agB1_out.opt()],
                )
                agB_outs = [agB0_out, agB1_out]
            # ---- epilogue ----
            if T % CHUNK != 0:
                nfin = T % CHUNK
                nc.sync.dma_start(out=y[:, T - nfin:T, :], in_=ybuf[:, 0:nfin, :])
            for d in range(2):
                t_ = hp.tile([128, 8, 128], MMD, tag=f"h{2 + d}")
                nc.sync.dma_start(
                    out=t_[:],
                    in_=agB_outs[d][:, 0:128].rearrange(
                        "(k p) b -> p k b", p=128),
                )
                h_prev[2 + d] = t_
            for cell in range(4):
                nc.sync.dma_start(out=hT_f[cell], in_=h_prev[cell][:])
                nc.sync.dma_start(out=c_f[cell], in_=c_prev[cell][:])
            nc.sync.dma_start(out=flag_f[:], in_=flag_prev[:])
            nc.sync.dma_start(out=idx_f[:], in_=idx[:])
    nc.finalize()
    return nc


def prep_inputs(h0, c0, w_ih0, w_hh0, b0, w_ih1, w_hh1, b1, lin_w, lin_b):
    """Host-side packing: per-core sliced/transposed weight + state arrays."""
    h0 = np.asarray(h0, np.float32).reshape(2, 2, B, H)
    c0 = np.asarray(c0, np.float32).reshape(2, 2, B, H)
    w_ih0 = np.asarray(w_ih0, np.float32)
    w_hh0 = np.asarray(w_hh0, np.float32)
    b0 = np.asarray(b0, np.float32)
    w_ih1 = np.asarray(w_ih1, np.float32)
    w_hh1 = np.asarray(w_hh1, np.float32)
    b1 = np.asarray(b1, np.float32)
    lin_w = np.asarray(lin_w, np.float32)
    lin_b = np.asarray(lin_b, np.float32)

    iota = np.broadcast_to((np.arange(V) - 100.0).astype(np.float32),
                           (128, V)).copy()
    nk = np.ones((128, V), np.float32)
    nk[:, KEEP_IDX] = 0.0

    hT0 = np.zeros((4, 128, 8, B), np.float32)
    for l in range(2):
        for d in range(2):
            cell = l * 2 + d
            hT0[cell] = h0[l, d].T.reshape(8, 128, B).transpose(1, 0, 2)

    in_maps = []
    for c in range(NCORES):
        rows = np.concatenate([np.arange(gb + c * 128, gb + c * 128 + 128)
                               for gb in GBASE])

        def packT(w, kt):
            # w: (4H, K*128) -> select rows -> [p, k, n]
            sel = w[rows, :]  # (512, kt*128)
            return np.ascontiguousarray(
                sel.reshape(512, kt, 128).transpose(2, 1, 0))

        w0T = np.stack([packT(w_hh0[d], 8) for d in range(2)])
        w1iT = np.stack([packT(w_ih1[d], 16) for d in range(2)])
        w1hT = np.stack([packT(w_hh1[d], 8) for d in range(2)])
        w0aug = np.stack([np.stack([w_ih0[d][rows, 0], b0[d][rows]])
                          for d in range(2)])
        b1row = np.stack([b1[d][rows][None, :] for d in range(2)])
        linTc = np.stack(
            [lin_w[:, c * 128:(c + 1) * 128].T,
             lin_w[:, H + c * 128:H + (c + 1) * 128].T], axis=1)
        c0slice = np.zeros((4, 128, 128), np.float32)
        for l in range(2):
            for d in range(2):
                c0slice[l * 2 + d] = c0[l, d][:, c * 128:(c + 1) * 128]
        in_maps.append({
            "w0T": tf32_round(np.ascontiguousarray(w0T)),
            "w0aug": tf32_round(np.ascontiguousarray(w0aug)),
            "w1iT": tf32_round(np.ascontiguousarray(w1iT)),
            "w1hT": tf32_round(np.ascontiguousarray(w1hT)),
            "b1row": tf32_round(np.ascontiguousarray(b1row)),
            "linTc": tf32_round(np.ascontiguousarray(linTc)),
            "linrow": tf32_round((lin_b / NCORES)[None, :]),
            "iotam": iota,
            "notkeep": nk,
            "hT0": tf32_round(hT0),
            "c0s": np.ascontiguousarray(c0slice),
            "onesrow": np.ones((1, 128), np.float32),
            "x0row": np.full((1, 128), MASK_IDX, np.float32),
            "flag0": np.zeros((128, 1), np.float32),
        })
    return in_maps


_NC_CACHE = {}


def _get_nc(T):
    if T not in _NC_CACHE:
        _NC_CACHE[T] = build(T)
    return _NC_CACHE[T]


T_LAUNCH = 512


def kernel(h0, c0, w_ih0, w_hh0, b0, w_ih1, w_hh1, b1, lin_w, lin_b,
           decoder_output_length, batch_size, _want_results=False):
    T = int(decoder_output_length)
    assert int(batch_size) == B
    in_maps = prep_inputs(h0, c0, w_ih0, w_hh0, b0, w_ih1, w_hh1, b1,
                          lin_w, lin_b)
    chunks = []
    t_done = 0
    res = None
    while t_done < T:
        t_this = min(T_LAUNCH, T - t_done)
        nc = _get_nc(t_this)
        res = bass_utils.run_bass_kernel_spmd(nc, in_maps,
                                              core_ids=list(range(NCORES)))
        chunks.append(res.results[0]["y"])
        t_done += t_this
        if t_done < T:
            idxs = res.results[0]["idx_f"]  # (128,1) float indices
            xrow = np.ascontiguousarray(idxs.reshape(1, 128))
            for c in range(NCORES):
                rc = res.results[c]
                in_maps[c] = dict(in_maps[c])
                in_maps[c]["hT0"] = rc["hT_f"]
                in_maps[c]["c0s"] = rc["c_f"]
                in_maps[c]["flag0"] = rc["flag_f"]
                in_maps[c]["x0row"] = xrow
    out = np.concatenate(chunks, axis=1) if len(chunks) > 1 else chunks[0]
    if _want_results:
        return out, res
    return out


# revision 26
# speedup vs baseline: 2.2233x; 1.0007x over previous
"""Trainium2 Bass kernel for nn_Decoder (2-layer bidirectional LSTM decoder,
autoregressive argmax feedback, T=512 steps, B=128, H=1024, V=64).

Strategy: 8-way tensor parallel over the 4H gate dimension. Each core holds a
512-wide slice of every gate projection (re-ordered [i,f,o,g] so activations
fuse), keeps the LSTM recurrence state resident, and exchanges state via five
collectives per step, scheduled back-to-back so that every piece of compute
hides inside some collective's window (collectives run on dedicated cores
concurrently with the engines):
  AR0    AllReduce of logit partial half 0 (lin_b/8 + dir-0 h1 term)
         -> its window covers the hh-d1 matmuls + dir-1 elementwise tail
  AR1    AllReduce of logit partial half 1 (dir-1 h1 term)
         -> its window covers next step's L0 h-recurrence partials
  AGH1a  AllGather of h1T dir-0 slices
         -> its window covers argmax -> x -> L0 close/elementwise/staging
  AGH0   AllGather of h0T(t+1) slices (both dirs)
         -> its window covers the hh-d0 matmuls
  AGH1b  AllGather of h1T dir-1 slices
         -> its window covers the 32 L1 input-projection matmuls + ew-d0
Logits are AR0out + AR1out. Device order is forced where the readiness-based
tile scheduler would invert it, via tiny DRAM->DRAM dependency-column DMAs
(AGH1a only becomes ready after AR1's input is staged; AGH1b only after
AGH0(t+1)'s input is staged). The bidirectional split works because each
LSTM direction's h-recurrence contracts only its own direction's h state.
Gathered-state DMA loads are split (igniter chunk first) so dependent matmul
groups become runnable progressively, keeping the PE p-state ramped.
"""

import os
import sys

import numpy as np

sys.path.insert(0, "/opt/trn_rl_repo")

import concourse.bass as bass  # noqa: E402
import concourse.mybir as mybir  # noqa: E402
import concourse.tile as tile  # noqa: E402
from concourse import bacc  # noqa: E402
from concourse import bass_utils  # noqa: E402
from concourse.masks import make_identity  # noqa: E402

H = 1024
V = 64
B = 128
NCORES = 8
MASK_IDX = 4.0
KEEP_IDX = 3
T_STEPS = int(os.environ.get("DEC_T", "512"))
CHUNK = int(os.environ.get("DEC_CHUNK", "8"))
MM_DT = mybir.dt.float32r if os.environ.get("DEC_MMDT", "fp32r") == "fp32r" else mybir.dt.float32
F32 = mybir.dt.float32
MMD = MM_DT
AF = mybir.ActivationFunctionType
ALU = mybir.AluOpType

# gate blocks packed per-core as [i, f, o, g] (torch order in rows is i,f,g,o)
GBASE = [0, H, 3 * H, 2 * H]

# h0T gathered-load split: chunk groups per input direction (pacing)
LOAD_GROUPS = [(0, 1), (1, 4), (4, 8)]


def tf32_round(x):
    if MM_DT == F32:
        return np.asarray(x, np.float32)
    xi = np.asarray(x, np.float32).view(np.uint32)
    xi = (xi + np.uint32(1 << 12)) & np.uint32(0xFFFFE000)
    return xi.view(np.float32)


def build(T=T_STEPS, final=True):
    nc = bacc.Bacc("TRN2", num_devices=NCORES)
    RG = [list(range(NCORES))]

    din = dict(kind="ExternalInput")
    w0T = nc.dram_tensor("w0T", [2, 128, 8, 512], MMD, **din)
    w0aug = nc.dram_tensor("w0aug", [2, 2, 512], MMD, **din)
    w1iT = nc.dram_tensor("w1iT", [2, 128, 16, 512], MMD, **din)
    w1hT = nc.dram_tensor("w1hT", [2, 128, 8, 512], MMD, **din)
    b1row = nc.dram_tensor("b1row", [2, 1, 512], MMD, **din)
    linTc = nc.dram_tensor("linTc", [128, 2, 64], MMD, **din)
    linrow = nc.dram_tensor("linrow", [1, 64], MMD, **din)
    iotam = nc.dram_tensor("iotam", [128, 64], F32, **din)
    notkeep = nc.dram_tensor("notkeep", [128, 64], F32, **din)
    hT0 = nc.dram_tensor("hT0", [4, 128, 8, 128], MMD, **din)
    c0s = nc.dram_tensor("c0s", [4, 128, 128], F32, **din)
    onesrow = nc.dram_tensor("onesrow", [1, 128], MMD, **din)
    x0row = nc.dram_tensor("x0row", [1, 128], MMD, **din)
    flag0 = nc.dram_tensor("flag0", [128, 1], F32, **din)
    hT_f = nc.dram_tensor("hT_f", [4, 128, 8, 128], MMD, kind="ExternalOutput")
    c_f = nc.dram_tensor("c_f", [4, 128, 128], F32, kind="ExternalOutput")
    flag_f = nc.dram_tensor("flag_f", [128, 1], F32, kind="ExternalOutput")
    idx_f = nc.dram_tensor("idx_f", [128, 1], F32, kind="ExternalOutput")
    y = nc.dram_tensor("y", [B, T, V], F32, kind="ExternalOutput")

    with tile.TileContext(nc) as tc:
        import contextlib

        ctx = contextlib.ExitStack()
        with ctx:
            wp = ctx.enter_context(tc.tile_pool(name="weights", bufs=1))
            hp = ctx.enter_context(tc.tile_pool(name="hstate", bufs=2))
            cp = ctx.enter_context(tc.tile_pool(name="cstate", bufs=2))
            gp = ctx.enter_context(tc.tile_pool(name="gact", bufs=1))
            ewp = ctx.enter_context(tc.tile_pool(name="ew", bufs=1))
            sp = ctx.enter_context(tc.tile_pool(name="send", bufs=2))
            ap_ = ctx.enter_context(tc.tile_pool(name="amax", bufs=2))
            yp = ctx.enter_context(tc.tile_pool(name="ybuf", bufs=1))
            pg = ctx.enter_context(tc.tile_pool(name="pgates", bufs=1, space="PSUM"))
            pt = ctx.enter_context(tc.tile_pool(name="ptrans", bufs=2, space="PSUM"))
            px = ctx.enter_context(tc.tile_pool(name="pmisc", bufs=1, space="PSUM"))
            dp = ctx.enter_context(tc.tile_pool(name="dram", bufs=2, space="DRAM"))

            # ---- load L0 weights + constants needed by the prologue.
            # The large L1/linear weight loads are deferred until after the
            # AGH0(0) issue so they stream under its collective window
            # instead of serializing ahead of the first step.
            w0_sb, w0a_sb = [], []
            for d in range(2):
                t_ = wp.tile([128, 8, 512], MMD, tag=f"w0_{d}")
                nc.sync.dma_start(out=t_[:], in_=w0T[d])
                w0_sb.append(t_)
                tb = wp.tile([1, 512], MMD, tag=f"w0b_{d}")
                nc.sync.dma_start(out=tb[:], in_=w0aug[d, 1:2])
                tx = wp.tile([1, 512], MMD, tag=f"w0x_{d}")
                nc.sync.dma_start(out=tx[:], in_=w0aug[d, 0:1])
                w0a_sb.append((tx, tb))
            ident = wp.tile([128, 128], F32, tag="ident")
            make_identity(nc, ident[:])
            ones = wp.tile([1, 128], MMD, tag="ones")
            nc.sync.dma_start(out=ones[:], in_=onesrow[:])

            # ---- initial state ----
            h_prev = []
            for cell in range(4):
                t_ = hp.tile([128, 8, 128], MMD, tag=f"h{cell}")
                nc.sync.dma_start(out=t_[:], in_=hT0[cell])
                h_prev.append(t_)
            c_prev = []
            for cell in range(4):
                t_ = cp.tile([128, 128], F32, tag=f"c{cell}")
                nc.sync.dma_start(out=t_[:], in_=c0s[cell])
                c_prev.append(t_)
            flag_prev = ap_.tile([128, 1], F32, tag="flag")
            nc.sync.dma_start(out=flag_prev[:], in_=flag0[:])
            x_row = ap_.tile([1, 128], MMD, tag="xrow")
            nc.sync.dma_start(out=x_row[:], in_=x0row[:])

            def lstm_ew_pre(g, c_in, cell):
                """gate PSUM [128,512] (i,f,o,g blocks) + c_in -> (cn, h2),
                Act/DVE only (no PE ops)."""
                a = gp.tile([128, 512], F32, tag=f"a{cell}")
                nc.scalar.activation(a[:, 0:384], g[:, 0:384], AF.Sigmoid)
                nc.scalar.activation(a[:, 384:512], g[:, 384:512], AF.Tanh)
                t1 = ewp.tile([128, 128], F32, tag=f"t1_{cell}")
                nc.vector.tensor_mul(t1[:], a[:, 128:256], c_in[:])
                t2 = ewp.tile([128, 128], F32, tag=f"t2_{cell}")
                nc.vector.tensor_mul(t2[:], a[:, 0:128], a[:, 384:512])
                cn = cp.tile([128, 128], F32, tag=f"c{cell}")
                nc.vector.tensor_add(cn[:], t1[:], t2[:])
                tc2 = ewp.tile([128, 128], F32, tag=f"tc2_{cell}")
                nc.scalar.activation(tc2[:], cn[:], AF.Tanh)
                h2 = gp.tile([128, 128], F32, tag=f"h2_{cell}")
                nc.vector.tensor_mul(h2[:], a[:, 256:384], tc2[:])
                return cn, h2

            def h_transpose(h2, dst, dst_col):
                ht = pt.tile([128, 128], F32, tag="ht")
                nc.tensor.transpose(ht[:], h2[:], ident[:])
                nc.vector.tensor_copy(dst[:, dst_col:dst_col + 128], ht[:])

            def lstm_ew(g, c_in, cell, dst, dst_col):
                cn, h2 = lstm_ew_pre(g, c_in, cell)
                h_transpose(h2, dst, dst_col)
                return cn

            # ---- prologue: L0(0) + stage + AGH0(0) ----
            g0 = []
            for d in range(2):
                g = pg.tile([128, 512], F32, tag=f"g0{d}")
                nc.tensor.matmul(g[:], (ones[:]), (w0a_sb[d][1][:]),
                                 start=True, stop=False)
                for k in range(8):
                    nc.tensor.matmul(g[:], (h_prev[d][:, k, :]),
                                     (w0_sb[d][:, k, :]),
                                     start=False, stop=False)
                nc.tensor.matmul(g[:], (x_row[:]), (w0a_sb[d][0][:]),
                                 start=False, stop=True)
                g0.append(g)
            sendA = sp.tile([128, 256], MMD, tag="sendA")
            agA_in = dp.tile([128, 256], MMD, tag="agAi")
            c_new = [None] * 4
            for d in range(2):
                c_new[d] = lstm_ew(g0[d], c_prev[d], d, sendA, d * 128)
                nc.sync.dma_start(out=agA_in[:, d * 128:(d + 1) * 128],
                                  in_=sendA[:, d * 128:(d + 1) * 128])
            c_prev = [c_new[0], c_new[1], c_prev[2], c_prev[3]]
            agA_out = dp.tile([1024, 256], MMD, tag="agAo", addr_space="Shared")
            nc.gpsimd.collective_compute(
                "AllGather", ALU.bypass, replica_groups=RG,
                ins=[agA_in.opt()], outs=[agA_out.opt()],
            )

            # ---- deferred L1/linear weight loads (hide under AGH0(0)) ----
            w1i_sb, w1h_sb, b1_sb = [], [], []
            for d in range(2):
                t_ = wp.tile([1, 512], MMD, tag=f"b1_{d}")
                nc.sync.dma_start(out=t_[:], in_=b1row[d])
                b1_sb.append(t_)
            for d in range(2):
                t_ = wp.tile([128, 8, 512], MMD, tag=f"w1h_{d}")
                nc.sync.dma_start(out=t_[:], in_=w1hT[d])
                w1h_sb.append(t_)
            for d in range(2):
                t_ = wp.tile([128, 16, 512], MMD, tag=f"w1i_{d}")
                nc.sync.dma_start(out=t_[:], in_=w1iT[d])
                w1i_sb.append(t_)
            lin_sb = wp.tile([128, 2, 64], MMD, tag="lin")
            nc.sync.dma_start(out=lin_sb[:], in_=linTc[:])
            linr_sb = wp.tile([1, 64], MMD, tag="linr")
            nc.sync.dma_start(out=linr_sb[:], in_=linrow[:])
            iot_sb = wp.tile([128, 64], F32, tag="iot")
            nc.sync.dma_start(out=iot_sb[:], in_=iotam[:])
            nk_sb = wp.tile([128, 64], F32, tag="nk")
            nc.sync.dma_start(out=nk_sb[:], in_=notkeep[:])

            idx = None
            ybuf = None
            agB_outs = [None, None]

            for t in range(T):
                last = t == T - 1
                # -- 1) h1T-d0(t-1) gathered load (skip t=0: prologue loaded)
                if t > 0:
                    t_ = hp.tile([128, 8, 128], MMD, tag="h2")
                    nc.sync.dma_start(
                        out=t_[:],
                        in_=agB_outs[0][:, 0:128].rearrange(
                            "(k p) b -> p k b", p=128),
                    )
                    h_prev[2] = t_
                # -- 2) g1(t) partials: bias both dirs + w_hh1 dir0
                #       (hh-d0 hides in the AGH0(t) window)
                g1 = []
                for d in range(2):
                    g = pg.tile([128, 512], F32, tag=f"g1{d}")
                    nc.tensor.matmul(g[:], (ones[:]), (b1_sb[d][:]),
                                     start=True, stop=False)
                    g1.append(g)
                for k in range(8):
                    nc.tensor.matmul(g1[0][:], (h_prev[2][:, k, :]),
                                     (w1h_sb[0][:, k, :]),
                                     start=False, stop=False)
                # -- 3) h0T(t) gathered loads, split for progressive pacing
                h0n = []
                for d in range(2):
                    t_ = hp.tile([128, 8, 128], MMD, tag=f"h{d}")
                    for k0, k1 in LOAD_GROUPS:
                        nc.sync.dma_start(
                            out=t_[:, k0:k1, :],
                            in_=agA_out[k0 * 128:k1 * 128,
                                        d * 128:(d + 1) * 128].rearrange(
                                "(k p) b -> p k b", p=128),
                        )
                    h0n.append(t_)
                    h_prev[d] = t_
                # -- 4) ih(t) for both output dirs (hides in AGH1b(t-1) win)
                for d in range(2):
                    for sd in range(2):
                        for k0, k1 in LOAD_GROUPS:
                            for k in range(k0, k1):
                                lastmm = d == 0 and sd == 1 and k == 7
                                nc.tensor.matmul(
                                    g1[d][:], (h0n[sd][:, k, :]),
                                    (w1i_sb[d][:, sd * 8 + k, :]),
                                    start=False, stop=lastmm,
                                )
                # -- 5a) ew-d0 + transpose + logit partial half 0 + AR0: all
                #        hide in the AGH1b(t-1) window (ready after ih stop)
                sendB = sp.tile([128, 256], MMD, tag="sendB")
                lpt = px.tile([128, 128], F32, tag="lp")
                lp0 = lpt[:, 0:64]
                lp1 = lpt[:, 64:128]
                nc.tensor.matmul(lp0[:], (ones[:]), (linr_sb[:]),
                                 start=True, stop=False)
                c_new[2], h2_0 = lstm_ew_pre(g1[0], c_prev[2], 2)
                h_transpose(h2_0, sendB, 0)
                nc.tensor.matmul(lp0[:], (sendB[:, 0:128]), (lin_sb[:, 0, :]),
                                 start=False, stop=True)
                skip_agh1 = final and last
                if not skip_agh1:
                    agB0_in = dp.tile([128, 129], MMD, tag="agB0i")
                    nc.sync.dma_start(out=agB0_in[:, 0:128],
                                      in_=sendB[:, 0:128])
                sendBl0 = sp.tile([128, 64], F32, tag="sendBl0")
                nc.vector.tensor_copy(sendBl0[:], lp0[:])
                ar0_in = dp.tile([128, 64], F32, tag="ar0i")
                nc.sync.dma_start(out=ar0_in[:], in_=sendBl0[:])
                ar0_out = dp.tile([128, 64], F32, tag="ar0o",
                                  addr_space="Shared")
                nc.gpsimd.collective_compute(
                    "AllReduce", ALU.add, replica_groups=RG,
                    ins=[ar0_in.opt()], outs=[ar0_out.opt()],
                )
                # -- 4b) h1T-d1(t-1) load (paced) + hh-d1(t) + ew-d1 + AR1:
                #        this tail hides in the AR0(t) window
                if t > 0:
                    t_ = hp.tile([128, 8, 128], MMD, tag="h3")
                    for k0, k1 in LOAD_GROUPS:
                        nc.sync.dma_start(
                            out=t_[:, k0:k1, :],
                            in_=agB_outs[1][k0 * 128:k1 * 128, 0:128].rearrange(
                                "(k p) b -> p k b", p=128),
                        )
                    h_prev[3] = t_
                for k0, k1 in LOAD_GROUPS:
                    for k in range(k0, k1):
                        nc.tensor.matmul(g1[1][:], (h_prev[3][:, k, :]),
                                         (w1h_sb[1][:, k, :]),
                                         start=False, stop=(k == 7))
                c_new[3], h2_1 = lstm_ew_pre(g1[1], c_prev[3], 3)
                h_transpose(h2_1, sendB, 128)
                nc.tensor.matmul(lp1[:], (sendB[:, 128:256]), (lin_sb[:, 1, :]),
                                 start=True, stop=True)
                c_prev = [c_prev[0], c_prev[1], c_new[2], c_new[3]]
                sendBl1 = sp.tile([128, 64], F32, tag="sendBl1")
                nc.vector.tensor_copy(sendBl1[:], lp1[:])
                ar1_in = dp.tile([128, 64], F32, tag="ar1i")
                nc.sync.dma_start(out=ar1_in[:], in_=sendBl1[:])
                if not skip_agh1:
                    agB1_in = dp.tile([128, 129], MMD, tag="agB1i")
                    nc.sync.dma_start(out=agB1_in[:, 0:128],
                                      in_=sendB[:, 128:256])
                # -- 7) AR1(t) + AGH1a(t). Device order forced by dep DMAs:
                # AR1 before AGH1a (d2d col from ar1_in), AGH1b after
                # AGH0(t+1) (d2d col from agA_in).
                ar1_out = dp.tile([128, 64], F32, tag="ar1o",
                                  addr_space="Shared")
                nc.gpsimd.collective_compute(
                    "AllReduce", ALU.add, replica_groups=RG,
                    ins=[ar1_in.opt()], outs=[ar1_out.opt()],
                )
                if not skip_agh1:
                    nc.sync.dma_start(out=agB0_in[:, 128:129].bitcast(F32),
                                      in_=ar1_in[:, 0:1])
                    agB0_out = dp.tile([1024, 129], MMD, tag="agB0o",
                                       addr_space="Shared")
                    nc.gpsimd.collective_compute(
                        "AllGather", ALU.bypass, replica_groups=RG,
                        ins=[agB0_in.opt()], outs=[agB0_out.opt()],
                    )
                    agB_outs = [agB0_out, None]
                # -- 8) g0(t+1) partials: bias + w_hh0 (hidden in AR window)
                if not last:
                    g0 = []
                    for d in range(2):
                        g = pg.tile([128, 512], F32, tag=f"g0{d}")
                        nc.tensor.matmul(g[:], (ones[:]), (w0a_sb[d][1][:]),
                                         start=True, stop=False)
                        for k in range(8):
                            nc.tensor.matmul(g[:], (h0n[d][:, k, :]),
                                             (w0_sb[d][:, k, :]),
                                             start=False, stop=False)
                        g0.append(g)
                # -- 9) logits load (both AR halves) + argmax chain
                L0h = ap_.tile([128, 64], F32, tag="L0h")
                nc.sync.dma_start(out=L0h[:], in_=ar0_out[:])
                L1h = ap_.tile([128, 64], F32, tag="L1h")
                nc.sync.dma_start(out=L1h[:], in_=ar1_out[:])
                L = ap_.tile([128, 64], F32, tag="L")
                nc.vector.tensor_add(L[:], L0h[:], L1h[:])
                m = ap_.tile([128, 1], F32, tag="m")
                nc.vector.tensor_reduce(m[:], L[:], axis=mybir.AxisListType.X,
                                        op=ALU.max)
                ismax = ap_.tile([128, 64], F32, tag="ismax")
                nc.vector.tensor_scalar(ismax[:], L[:], m[:], None,
                                        op0=ALU.is_ge)
                cand = ap_.tile([128, 64], F32, tag="cand")
                nc.vector.tensor_mul(cand[:], ismax[:], iot_sb[:])
                idxm = ap_.tile([128, 1], F32, tag="idxm")
                nc.vector.tensor_reduce(idxm[:], cand[:],
                                        axis=mybir.AxisListType.X, op=ALU.min)
                idx = ap_.tile([128, 1], F32, tag="idx")
                nc.vector.tensor_scalar(idx[:], idxm[:], 100.0, None,
                                        op0=ALU.add)
                # -- 10) x feedback + close g0(t+1) (hidden in AGH1 window)
                if not last:
                    x_ps = px.tile([1, 128], F32, tag="xps")
                    nc.tensor.transpose(x_ps[:], idx[:], ident[:])
                    x_row = ap_.tile([1, 128], MMD, tag="xrow")
                    nc.vector.tensor_copy(x_row[:], x_ps[:])
                    for d in range(2):
                        nc.tensor.matmul(g0[d][:], (x_row[:]),
                                         (w0a_sb[d][0][:]),
                                         start=False, stop=True)
                # -- 11) flag + masked store
                flagb = ap_.tile([128, 1], F32, tag="flagb")
                nc.vector.tensor_scalar(flagb[:], idx[:], 1.0, None,
                                        op0=ALU.is_equal)
                fnew = ap_.tile([128, 1], F32, tag="flag")
                nc.vector.tensor_max(fnew[:], flag_prev[:], flagb[:])
                tk = ap_.tile([128, 64], F32, tag="tk")
                nc.vector.tensor_mul(tk[:], L[:], nk_sb[:])
                tk2 = ap_.tile([128, 64], F32, tag="tk2")
                nc.vector.tensor_scalar(tk2[:], tk[:], fnew[:], None,
                                        op0=ALU.mult)
                if t % CHUNK == 0:
                    ybuf = yp.tile([128, CHUNK, 64], F32, tag="ybuf")
                nc.vector.tensor_sub(ybuf[:, t % CHUNK, :], L[:], tk2[:])
                if t % CHUNK == CHUNK - 1:
                    nc.sync.dma_start(out=y[:, t - CHUNK + 1:t + 1, :],
                                      in_=ybuf[:])
                flag_prev = fnew
                # -- 12) L0(t+1) elementwise + stage + AGH0(t+1) + AGH1b(t)
                if not last:
                    sendA = sp.tile([128, 256], MMD, tag="sendA")
                    agA_in = dp.tile([128, 256], MMD, tag="agAi")
                    for d in range(2):
                        c_new[d] = lstm_ew(g0[d], c_prev[d], d, sendA, d * 128)
                    nc.sync.dma_start(out=agA_in[:], in_=sendA[:])
                    c_prev = [c_new[0], c_new[1], c_prev[2], c_prev[3]]
                    agA_out = dp.tile([1024, 256], MMD, tag="agAo",
                                      addr_space="Shared")
                    nc.gpsimd.collective_compute(
                        "AllGather", ALU.bypass, replica_groups=RG,
                        ins=[agA_in.opt()], outs=[agA_out.opt()],
                    )
                    # dep col: AGH1b(t) becomes ready only after agA(t+1)
                    nc.sync.dma_start(out=agB1_in[:, 128:129],
                                      in_=agA_in[:, 0:1])
                elif not skip_agh1:
                    nc.sync.dma_start(out=agB1_in[:, 128:129].bitcast(F32),
                                      in_=ar1_in[:, 0:1])
                if not skip_agh1:
                    agB1_out = dp.tile([1024, 129], MMD, tag="agB1o",
                                       addr_space="Shared")
                    nc.gpsimd.collective_compute(
                        "AllGather", ALU.bypass, replica_groups=RG,
                        ins=[agB1_in.opt()], outs=[agB1_out.opt()],
                    )
                    agB_outs = [agB0_out, agB1_out]
            # ---- epilogue ----
            if T % CHUNK != 0:
                nfin = T % CHUNK
                nc.sync.dma_start(out=y[:, T - nfin:T, :], in_=ybuf[:, 0:nfin, :])
            if not final:
                for d in range(2):
                    t_ = hp.tile([128, 8, 128], MMD, tag=f"h{2 + d}")
                    nc.sync.dma_start(
                        out=t_[:],
                        in_=agB_outs[d][:, 0:128].rearrange(
                            "(k p) b -> p k b", p=128),
                    )
                    h_prev[2 + d] = t_
            ncell = 2 if final else 4
            for cell in range(ncell):
                nc.sync.dma_start(out=hT_f[cell], in_=h_prev[cell][:])
            for cell in range(4):
                nc.sync.dma_start(out=c_f[cell], in_=c_prev[cell][:])
            nc.sync.dma_start(out=flag_f[:], in_=flag_prev[:])
            nc.sync.dma_start(out=idx_f[:], in_=idx[:])
    nc.finalize()
    return nc


def prep_inputs(h0, c0, w_ih0, w_hh0, b0, w_ih1, w_hh1, b1, lin_w, lin_b):
    """Host-side packing: per-core sliced/transposed weight + state arrays."""
    h0 = np.asarray(h0, np.float32).reshape(2, 2, B, H)
    c0 = np.asarray(c0, np.float32).reshape(2, 2, B, H)
    w_ih0 = np.asarray(w_ih0, np.float32)
    w_hh0 = np.asarray(w_hh0, np.float32)
    b0 = np.asarray(b0, np.float32)
    w_ih1 = np.asarray(w_ih1, np.float32)
    w_hh1 = np.asarray(w_hh1, np.float32)
    b1 = np.asarray(b1, np.float32)
    lin_w = np.asarray(lin_w, np.float32)
    lin_b = np.asarray(lin_b, np.float32)

    iota = np.broadcast_to((np.arange(V) - 100.0).astype(np.float32),
                           (128, V)).copy()
    nk = np.ones((128, V), np.float32)
    nk[:, KEEP_IDX] = 0.0

    hT0 = np.zeros((4, 128, 8, B), np.float32)
    for l in range(2):
        for d in range(2):
            cell = l * 2 + d
            hT0[cell] = h0[l, d].T.reshape(8, 128, B).transpose(1, 0, 2)

    in_maps = []
    for c in range(NCORES):
        rows = np.concatenate([np.arange(gb + c * 128, gb + c * 128 + 128)
                               for gb in GBASE])

        def packT(w, kt):
            # w: (4H, K*128) -> select rows -> [p, k, n]
            sel = w[rows, :]  # (512, kt*128)
            return np.ascontiguousarray(
                sel.reshape(512, kt, 128).transpose(2, 1, 0))

        w0T = np.stack([packT(w_hh0[d], 8) for d in range(2)])
        w1iT = np.stack([packT(w_ih1[d], 16) for d in range(2)])
        w1hT = np.stack([packT(w_hh1[d], 8) for d in range(2)])
        w0aug = np.stack([np.stack([w_ih0[d][rows, 0], b0[d][rows]])
                          for d in range(2)])
        b1row = np.stack([b1[d][rows][None, :] for d in range(2)])
        linTc = np.stack(
            [lin_w[:, c * 128:(c + 1) * 128].T,
             lin_w[:, H + c * 128:H + (c + 1) * 128].T], axis=1)
        c0slice = np.zeros((4, 128, 128), np.float32)
        for l in range(2):
            for d in range(2):
                c0slice[l * 2 + d] = c0[l, d][:, c * 128:(c + 1) * 128]
        in_maps.append({
            "w0T": tf32_round(np.ascontiguousarray(w0T)),
            "w0aug": tf32_round(np.ascontiguousarray(w0aug)),
            "w1iT": tf32_round(np.ascontiguousarray(w1iT)),
            "w1hT": tf32_round(np.ascontiguousarray(w1hT)),
            "b1row": tf32_round(np.ascontiguousarray(b1row)),
            "linTc": tf32_round(np.ascontiguousarray(linTc)),
            "linrow": tf32_round((lin_b / NCORES)[None, :]),
            "iotam": iota,
            "notkeep": nk,
            "hT0": tf32_round(hT0),
            "c0s": np.ascontiguousarray(c0slice),
            "onesrow": np.ones((1, 128), np.float32),
            "x0row": np.full((1, 128), MASK_IDX, np.float32),
            "flag0": np.zeros((128, 1), np.float32),
        })
    return in_maps


_NC_CACHE = {}


def _get_nc(T, final=True):
    key = (T, final)
    if key not in _NC_CACHE:
        _NC_CACHE[key] = build(T, final)
    return _NC_CACHE[key]


T_LAUNCH = 512


def kernel(h0, c0, w_ih0, w_hh0, b0, w_ih1, w_hh1, b1, lin_w, lin_b,
           decoder_output_length, batch_size, _want_results=False):
    T = int(decoder_output_length)
    assert int(batch_size) == B
    in_maps = prep_inputs(h0, c0, w_ih0, w_hh0, b0, w_ih1, w_hh1, b1,
                          lin_w, lin_b)
    chunks = []
    t_done = 0
    res = None
    while t_done < T:
        t_this = min(T_LAUNCH, T - t_done)
        nc = _get_nc(t_this, final=(t_done + t_this >= T))
        res = bass_utils.run_bass_kernel_spmd(nc, in_maps,
                                              core_ids=list(range(NCORES)))
        chunks.append(res.results[0]["y"])
        t_done += t_this
        if t_done < T:
            idxs = res.results[0]["idx_f"]  # (128,1) float indices
            xrow = np.ascontiguousarray(idxs.reshape(1, 128))
            for c in range(NCORES):
                rc = res.results[c]
                in_maps[c] = dict(in_maps[c])
                in_maps[c]["hT0"] = rc["hT_f"]
                in_maps[c]["c0s"] = rc["c_f"]
                in_maps[c]["flag0"] = rc["flag_f"]
                in_maps[c]["x0row"] = xrow
    out = np.concatenate(chunks, axis=1) if len(chunks) > 1 else chunks[0]
    if _want_results:
        return out, res
    return out
